# revision 7
# baseline (speedup 1.0000x reference)
"""Trainium2 Bass kernel for nn_BasicNCAModel (neural cellular automaton).

Strategy (pure data parallelism, batch 16 -> 2 images per core x 8 cores):

* State layout per core: [128 partitions = 2 images x 64 channels,
  130 x 130 reflect-padded grid] in SBUF fp16, ping-pong buffered.
* The two depthwise 3x3 convs are folded into the hidden matmul:
  h = relu(sum_tap E_tap @ x_shift(tap) + b) with E_tap[256, 64].
* fp8 DoubleRow: NPAIRS of the 9 taps run as e4m3 DoubleRow matmuls that
  contract TWO taps per instruction (2 fp8 weights/PE cell).  The moving
  operand comes from packed fp8 state copies x8[b-idx][130 rows, 128 cols]
  (H-stride 128, no column halo) so a group's 512 pixels are contiguous;
  the K-pair stride (delta = d_bidx*16640 + d_a*128) is 16B-aligned as
  DoubleRow requires.  Remaining taps (incl. the W0-carrying center) stay
  fp16 for accuracy; per-hidden-row scales (folded into bias and w_final)
  keep the e4m3 weights in range.
* No per-step barrier: halo cols are refreshed per group, halo rows right
  after the first/last group of each step, so consecutive steps pipeline
  on the PE without HAM re-throttle.
* Stochastic fire gate (pre-merged with the static life mask on the host)
  is broadcast per group on GpSimd and applied on DVE; the masked initial
  state x0*L feeds the first update so step 0 needs no life tiles.
"""
import sys
sys.path.insert(0, '/opt/trn_rl_repo')

import numpy as np

B, H, W, C = 16, 128, 128, 64
HID = 256
STEPS = 8
NCORES = 8
BPC = B // NCORES            # images per core = 2
WP, HP = W + 2, H + 2        # padded grid 130 x 130
RPG = 4                      # W-rows per group
NPIX = RPG * H               # 512 pixels per matmul tile
NG = W // RPG                # 32 groups per step
CSZ = WP * H                 # 16640 elements per packed fp8 copy

NPAIRS = 3                   # fp8 DoubleRow tap pairs (0, 2 or 3)

# tap schedule: pairs are e4m3 DoubleRow (2 taps/MM); singles + center fp16.
# b-copies: bidx 0 <-> b=0 (cols 0:128 of padded state), 1 <-> b=2 (cols 2:130)
BIDX = {0: 0, 2: 1}
if NPAIRS == 3:
    PAIRS = [((0, 0), (1, 0)), ((2, 0), (0, 2)), ((1, 2), (2, 2))]
    SINGLES = [(0, 1), (2, 1), (1, 1)]
elif NPAIRS == 2:
    PAIRS = [((0, 0), (1, 0)), ((1, 2), (2, 2))]
    SINGLES = [(2, 0), (0, 2), (0, 1), (2, 1), (1, 1)]
else:
    PAIRS = []
    SINGLES = [(a, b) for a in range(3) for b in range(3)]
NS = len(SINGLES)

_nc_cache = {}


def _build():
    import concourse.bacc as bacc
    import concourse.mybir as mybir
    import concourse.tile as tile
    from concourse.bass import AP

    F32 = mybir.dt.float32
    F16 = mybir.dt.float16
    BF16 = mybir.dt.bfloat16
    F8 = mybir.dt.float8e4
    AF = mybir.ActivationFunctionType
    ALU = mybir.AluOpType
    DR = mybir.MatmulPerfMode.DoubleRow

    nc = bacc.Bacc("TRN2", target_bir_lowering=False, debug=False,
                   enable_asserts=False, num_devices=NCORES)

    X0 = nc.dram_tensor("x0", [128, WP, HP], F16, kind="ExternalInput")
    X0L = nc.dram_tensor("x0l", [128, WP, HP], F16, kind="ExternalInput")
    X8 = nc.dram_tensor("x8", [128, 2, WP, H], F8, kind="ExternalInput")
    WT8 = nc.dram_tensor("wt8", [128, max(NPAIRS, 1), 2, 2, 128], F8,
                         kind="ExternalInput")
    WTB = nc.dram_tensor("wtb", [128, NS, 2, 128], F16, kind="ExternalInput")
    WF = nc.dram_tensor("wf", [128, 2, 64], BF16, kind="ExternalInput")
    BI = nc.dram_tensor("bi", [128, 2], F32, kind="ExternalInput")
    GL = nc.dram_tensor("gl", [STEPS, NG, 2, NPIX], BF16, kind="ExternalInput")
    OUT = nc.dram_tensor("out", [128, W, H], F16, kind="ExternalOutput")

    with tile.TileContext(nc) as tc:
        with tc.tile_pool(name="const", bufs=1) as cp, \
             tc.tile_pool(name="hbuf", bufs=2) as hp, \
             tc.tile_pool(name="gbuf", bufs=3) as gp, \
             tc.tile_pool(name="ph", bufs=1, space="PSUM") as php, \
             tc.tile_pool(name="pdx", bufs=2, space="PSUM") as pdxp:

            xA = cp.tile([128, WP, HP], F16, tag="xA")
            xB = cp.tile([128, WP, HP], F16, tag="xB")
            x0l = cp.tile([128, WP, HP], F16, tag="x0l")
            x8A = cp.tile([128, 2, WP, H], F8, tag="x8A")
            x8B = cp.tile([128, 2, WP, H], F8, tag="x8B")
            wt8 = cp.tile([128, max(NPAIRS, 1), 2, 2, 128], F8, tag="wt8")
            wtb = cp.tile([128, NS, 2, 128], F16, tag="wtb")
            wf = cp.tile([128, 2, 64], BF16, tag="wf")
            bi = cp.tile([128, 2], F32, tag="bi")

            for c in range(4):
                r0, r1 = (WP * c) // 4, (WP * (c + 1)) // 4
                nc.sync.dma_start(xA[:, r0:r1, :], X0[:, r0:r1, :])
                nc.sync.dma_start(x0l[:, r0:r1, :], X0L[:, r0:r1, :])
                nc.sync.dma_start(x8A[:, :, r0:r1, :], X8[:, :, r0:r1, :])
            nc.sync.dma_start(wt8[:], WT8[:])
            nc.sync.dma_start(wtb[:], WTB[:])
            nc.sync.dma_start(wf[:], WF[:])
            nc.sync.dma_start(bi[:], BI[:])

            def dr_rhs(x8s, img, w0, t1, t2):
                """[64, 2@delta, 512@1] moving AP for a DoubleRow tap pair."""
                (a1, b1), (a2, b2) = t1, t2
                delta = (BIDX[b2] - BIDX[b1]) * CSZ + (a2 - a1) * H
                assert delta > 0 and delta % 16 == 0, (t1, t2, delta)
                base = x8s[img * 64:(img + 1) * 64, BIDX[b1],
                           w0 + a1:w0 + a1 + RPG, :]
                ap = [list(base.ap[0]), [delta, 2], [1, NPIX]]
                return AP(base.tensor, base.offset, ap)

            def emit_tail(p):
                """mm2 + gate + state update + fp8 copy refresh for a
                finished group (issued one group later: PE never stalls)."""
                hA, hB, gate, gateB, xs, xd, x8d, w0, t = p
                first, last = t == 0, t == STEPS - 1
                dx = pdxp.tile([128, NPIX], F32, tag="dx")
                for k in range(2):
                    nc.tensor.matmul(dx[0:64, :], wf[:, k, :], hA[:, k, :],
                                     start=k == 0, stop=k == 1,
                                     skip_group_check=True)
                    nc.tensor.matmul(dx[64:128, :], wf[:, k, :], hB[:, k, :],
                                     start=k == 0, stop=k == 1,
                                     skip_group_check=True,
                                     tile_position=(0, 64))
                tg = hp.tile([128, NPIX], F16, tag="tg")
                nc.vector.tensor_tensor(tg[0:64, :], dx[0:64, :],
                                        gate[0:64, :], ALU.mult)
                nc.vector.tensor_tensor(tg[64:128, :], dx[64:128, :],
                                        gateB[64:128, :], ALU.mult)
                tg3 = tg[:].rearrange("p (a b) -> p a b", a=RPG)
                rows = slice(w0 + 1, w0 + 1 + RPG)
                src = x0l if first else xs
                nc.gpsimd.tensor_tensor(xd[:, rows, 1:1 + H], tg3,
                                        src[:, rows, 1:1 + H], ALU.add)
                # column halos (reflect) for this group's rows
                nc.vector.tensor_copy(xd[:, rows, 0], xd[:, rows, 2])
                nc.vector.tensor_copy(xd[:, rows, HP - 1], xd[:, rows, HP - 3])
                # packed fp8 b-copies (b=0 needs col halo 0; b=2 needs 129)
                nc.scalar.copy(x8d[:, 0, rows, :], xd[:, rows, 0:H])
                nc.vector.tensor_copy(x8d[:, 1, rows, :], xd[:, rows, 2:2 + H])
                if w0 == 0:
                    # row halos for next step's first group (needs rows 0..5)
                    nc.vector.tensor_copy(xd[:, 0, :], xd[:, 2, :])
                    nc.scalar.copy(x8d[:, 0, 0, :], xd[:, 0, 0:H])
                    nc.vector.tensor_copy(x8d[:, 1, 0, :], xd[:, 0, 2:2 + H])
                if w0 == W - RPG:
                    nc.vector.tensor_copy(xd[:, WP - 1, :], xd[:, WP - 3, :])
                    nc.scalar.copy(x8d[:, 0, WP - 1, :], xd[:, WP - 1, 0:H])
                    nc.vector.tensor_copy(x8d[:, 1, WP - 1, :],
                                          xd[:, WP - 1, 2:2 + H])
                if last:
                    nc.sync.dma_start(OUT[:, w0:w0 + RPG, :],
                                      xd[:, rows, 1:1 + H])

            pend = None
            for t in range(STEPS):
                xs, xd = (xA, xB) if t % 2 == 0 else (xB, xA)
                x8s, x8d = (x8A, x8B) if t % 2 == 0 else (x8B, x8A)
                for g in range(NG):
                    w0 = RPG * g

                    glA = gp.tile([1, NPIX], BF16, tag="glA")
                    glB = gp.tile([1, NPIX], BF16, tag="glB")
                    nc.sync.dma_start(glA[:], GL[t, g, 0:1, :])
                    nc.sync.dma_start(glB[:], GL[t, g, 1:2, :])
                    gate = gp.tile([128, NPIX], BF16, tag="gate")
                    gateB = gp.tile([128, NPIX], BF16, tag="gateB")
                    nc.gpsimd.partition_broadcast(gate[:, :], glA[:])
                    nc.gpsimd.partition_broadcast(gateB[:, :], glB[:])

                    phs = [[php.tile([128, NPIX], F32, tag=f"ph{im}{m}",
                                     name=f"ph{im}{m}")
                            for m in range(2)] for im in range(2)]
                    hA = hp.tile([128, 2, NPIX], BF16, tag="hA")
                    hB = hp.tile([128, 2, NPIX], BF16, tag="hB")
                    for m in range(2):
                        nmm = NPAIRS + NS
                        mi = 0
                        for pi, (t1, t2) in enumerate(PAIRS):
                            st, sp = mi == 0, mi == nmm - 1
                            for im in range(2):
                                nc.tensor.matmul(
                                    phs[im][m][:], wt8[im * 64:(im + 1) * 64, pi, m],
                                    dr_rhs(x8s, im, w0, t1, t2),
                                    start=st, stop=sp, perf_mode=DR,
                                    skip_group_check=True)
                            mi += 1
                        for si, (a, b) in enumerate(SINGLES):
                            st, sp = mi == 0, mi == nmm - 1
                            for im in range(2):
                                rhs = xs[im * 64:(im + 1) * 64,
                                         w0 + a:w0 + a + RPG, b:b + H]
                                nc.tensor.matmul(
                                    phs[im][m][:], wtb[im * 64:(im + 1) * 64, si, m],
                                    rhs, start=st, stop=sp,
                                    skip_group_check=True)
                            mi += 1
                        # relu + bias, PSUM -> SBUF bf16 (3 on ACT, 1 on DVE)
                        nc.scalar.activation(hA[:, m, :], phs[0][m][:], AF.Relu,
                                             bias=bi[:, m:m + 1])
                        nc.scalar.activation(hB[:, m, :], phs[1][m][:],
                                             AF.Relu, bias=bi[:, m:m + 1])
                        if m == 0 and pend is not None:
                            emit_tail(pend)
                            pend = None

                    if pend is not None:
                        emit_tail(pend)
                    pend = (hA, hB, gate, gateB, xs, xd, x8d, w0, t)

            emit_tail(pend)

    nc.compile()
    return nc


def _host_pack(x, w_conv1, w_conv2, w_hidden, b_hidden, w_final, rand_vals):
    import ml_dtypes
    bf16 = ml_dtypes.bfloat16
    f16 = np.float16
    e4m3 = ml_dtypes.float8_e4m3

    Wh = np.asarray(w_hidden, np.float64)            # [256, 192]
    w1 = np.asarray(w_conv1, np.float64)[:, 0]       # [64, 3, 3]
    w2 = np.asarray(w_conv2, np.float64)[:, 0]

    E = {}
    for a in range(3):
        for b in range(3):
            Et = Wh[:, 64:128] * w1[None, :, a, b] + Wh[:, 128:192] * w2[None, :, a, b]
            if (a, b) == (1, 1):
                Et = Et + Wh[:, 0:64]
            E[(a, b)] = Et                            # [256, 64]

    fp8taps = [tp for pr in PAIRS for tp in pr]
    if fp8taps:
        rowmax = np.max(np.stack([np.abs(E[tp]) for tp in fp8taps]), axis=(0, 2))
        s = np.clip(224.0 / np.maximum(rowmax, 1e-6), 0.25, 4096.0)   # [256]
    else:
        s = np.ones(HID)

    wt8 = np.zeros((128, max(NPAIRS, 1), 2, 2, 128), np.float32)
    for pi, (t1, t2) in enumerate(PAIRS):
        for ko, tp in enumerate((t1, t2)):
            Es = E[tp] * s[:, None]                   # [256, 64]
            for m in range(2):
                lhsT = Es[128 * m:128 * (m + 1), :].T                 # [64, 128]
                wt8[0:64, pi, m, ko, :] = lhsT
                wt8[64:128, pi, m, ko, :] = lhsT
    wt8 = wt8.astype(e4m3)

    wtb = np.zeros((128, NS, 2, 128), np.float32)
    for si, tp in enumerate(SINGLES):
        Es = E[tp] * s[:, None]
        for m in range(2):
            lhsT = Es[128 * m:128 * (m + 1), :].T
            wtb[0:64, si, m, :] = lhsT
            wtb[64:128, si, m, :] = lhsT
    wtb = wtb.astype(f16)

    bv = np.asarray(b_hidden, np.float64) * s
    bi = np.stack([bv[0:128], bv[128:256]], axis=1).astype(np.float32)

    wfz = np.asarray(w_final, np.float64).copy()     # [64, 256]
    wfz[0:4, :] = 0.0                                # immutable image channels
    wfT = (wfz / s[None, :]).T                       # [256, 64]
    wf = np.ascontiguousarray(
        np.stack([wfT[0:128], wfT[128:256]], axis=1)).astype(bf16)

    # life mask is static: channel-0 updates masked out -> life == (x0 > 0)
    Lhw = np.asarray(x)[..., 0] > 0                  # [B, H, W]
    Lwh = np.ascontiguousarray(Lhw.transpose(0, 2, 1))   # [B, W, H]
    G = np.asarray(rand_vals)[..., 0] > 0.5          # [S, B, H, W]
    GLw = G.transpose(0, 1, 3, 2) & Lwh[None]        # [S, B, W, H]

    x_chw = np.asarray(x, np.float32).transpose(0, 3, 2, 1)      # [B, C, W, H]
    xp = np.pad(x_chw, ((0, 0), (0, 0), (1, 1), (1, 1)), mode='reflect')
    xp = xp.astype(f16)
    xl = x_chw * Lwh[:, None].astype(np.float32)
    xpl = np.pad(xl, ((0, 0), (0, 0), (1, 1), (1, 1)),
                 mode='reflect').astype(f16)

    in_maps = []
    for i in range(NCORES):
        sl = slice(BPC * i, BPC * (i + 1))
        x0 = np.ascontiguousarray(xp[sl].reshape(BPC * C, WP, HP))
        x0l = np.ascontiguousarray(xpl[sl].reshape(BPC * C, WP, HP))
        x8 = np.stack([x0[:, :, 0:H], x0[:, :, 2:2 + H]], axis=1)
        x8 = np.ascontiguousarray(x8).astype(e4m3)
        glc = np.ascontiguousarray(
            GLw[:, sl].reshape(STEPS, BPC, NG, NPIX).transpose(0, 2, 1, 3)
        ).astype(bf16)
        in_maps.append({
            "x0": x0, "x0l": x0l, "x8": x8,
            "wt8": wt8, "wtb": wtb, "wf": wf, "bi": bi, "gl": glc,
        })
    return in_maps


def _run(inputs, trace=False, trace_kwargs=None):
    from concourse.bass_utils import run_bass_kernel_spmd
    if "nc" not in _nc_cache:
        _nc_cache["nc"] = _build()
    nc = _nc_cache["nc"]
    in_maps = _host_pack(
        inputs["x"], inputs["w_conv1"], inputs["w_conv2"], inputs["w_hidden"],
        inputs["b_hidden"], inputs["w_final"], inputs["rand_vals"])
    kwargs = {}
    if trace:
        kwargs["trace"] = True
        if trace_kwargs:
            kwargs.update(trace_kwargs)
    res = run_bass_kernel_spmd(nc, in_maps, core_ids=list(range(NCORES)), **kwargs)
    outs = []
    for i in range(NCORES):
        o = res.results[i]["out"].astype(np.float32).reshape(BPC, C, W, H)
        outs.append(o.transpose(0, 3, 2, 1))         # -> [b, H, W, C]
    full = np.concatenate(outs, axis=0).astype(np.float32)
    return full, res


def kernel(**inputs) -> np.ndarray:
    steps = int(np.asarray(inputs.get("steps", STEPS)))
    assert steps == STEPS, f"kernel compiled for {STEPS} steps, got {steps}"
    out, _ = _run(inputs)
    return out


# revision 8
# speedup vs baseline: 3.1656x; 3.1656x over previous
"""Trainium2 Bass kernel for nn_BasicNCAModel (neural cellular automaton).

Strategy (pure data parallelism, batch 16 -> 2 images per core x 8 cores):

* State layout per core: [128 partitions = 2 images x 64 channels,
  130 x 130 reflect-padded grid] in SBUF fp16, ping-pong buffered.
* The two depthwise 3x3 convs are folded into the hidden matmul:
  h = relu(sum_tap E_tap @ x_shift(tap) + b) with E_tap[256, 64].
* fp8 DoubleRow: NPAIRS of the 9 taps run as e4m3 DoubleRow matmuls that
  contract TWO taps per instruction (2 fp8 weights/PE cell).  The moving
  operand comes from packed fp8 state copies x8[b-idx][130 rows, 128 cols]
  (H-stride 128, no column halo) so a group's 512 pixels are contiguous;
  the K-pair stride (delta = d_bidx*16640 + d_a*128) is 16B-aligned as
  DoubleRow requires.  Remaining taps (incl. the W0-carrying center) stay
  fp16 for accuracy; per-hidden-row scales (folded into bias and w_final)
  keep the e4m3 weights in range.
* No per-step barrier: halo cols are refreshed per group, halo rows right
  after the first/last group of each step, so consecutive steps pipeline
  on the PE without HAM re-throttle.
* Stochastic fire gate (pre-merged with the static life mask on the host)
  is broadcast per group on GpSimd and applied on DVE; the masked initial
  state x0*L feeds the first update so step 0 needs no life tiles.
"""
import sys
sys.path.insert(0, '/opt/trn_rl_repo')

import numpy as np

B, H, W, C = 16, 128, 128, 64
HID = 256
STEPS = 8
NCORES = 8
BPC = B // NCORES            # images per core = 2
WP, HP = W + 2, H + 2        # padded grid 130 x 130
RPG = 4                      # W-rows per group
NPIX = RPG * H               # 512 pixels per matmul tile
NG = W // RPG                # 32 groups per step
CSZ = WP * H                 # 16640 elements per packed fp8 copy

NPAIRS = 3                   # fp8 DoubleRow tap pairs (0, 2 or 3)

# tap schedule: pairs are e4m3 DoubleRow (2 taps/MM); singles + center fp16.
# b-copies: bidx 0 <-> b=0 (cols 0:128 of padded state), 1 <-> b=2 (cols 2:130)
BIDX = {0: 0, 2: 1}
if NPAIRS == 3:
    PAIRS = [((0, 0), (1, 0)), ((2, 0), (0, 2)), ((1, 2), (2, 2))]
    SINGLES = [(0, 1), (2, 1), (1, 1)]
elif NPAIRS == 2:
    PAIRS = [((0, 0), (1, 0)), ((1, 2), (2, 2))]
    SINGLES = [(2, 0), (0, 2), (0, 1), (2, 1), (1, 1)]
else:
    PAIRS = []
    SINGLES = [(a, b) for a in range(3) for b in range(3)]
NS = len(SINGLES)

_nc_cache = {}


def _build():
    import concourse.bacc as bacc
    import concourse.mybir as mybir
    import concourse.tile as tile
    from concourse.bass import AP

    F32 = mybir.dt.float32
    F16 = mybir.dt.float16
    BF16 = mybir.dt.bfloat16
    F8 = mybir.dt.float8e4
    AF = mybir.ActivationFunctionType
    ALU = mybir.AluOpType
    DR = mybir.MatmulPerfMode.DoubleRow

    nc = bacc.Bacc("TRN2", target_bir_lowering=False, debug=False,
                   enable_asserts=False, num_devices=NCORES)

    X0 = nc.dram_tensor("x0", [128, WP, HP], F16, kind="ExternalInput")
    X0L = nc.dram_tensor("x0l", [128, WP, HP], F16, kind="ExternalInput")
    X8 = nc.dram_tensor("x8", [128, 2, WP, H], F8, kind="ExternalInput")
    WT8 = nc.dram_tensor("wt8", [128, max(NPAIRS, 1), 2, 2, 128], F8,
                         kind="ExternalInput")
    WTB = nc.dram_tensor("wtb", [128, NS, 2, 128], F16, kind="ExternalInput")
    WF = nc.dram_tensor("wf", [128, 2, 64], BF16, kind="ExternalInput")
    BI = nc.dram_tensor("bi", [128, 2], F32, kind="ExternalInput")
    GL = nc.dram_tensor("gl", [STEPS, NG, 2, NPIX], BF16, kind="ExternalInput")
    OUT = nc.dram_tensor("out", [128, W, H], F16, kind="ExternalOutput")

    with tile.TileContext(nc) as tc:
        with tc.tile_pool(name="const", bufs=1) as cp, \
             tc.tile_pool(name="hbuf", bufs=2) as hp, \
             tc.tile_pool(name="gbuf", bufs=3) as gp, \
             tc.tile_pool(name="ph", bufs=1, space="PSUM") as php, \
             tc.tile_pool(name="pdx", bufs=2, space="PSUM") as pdxp:

            xA = cp.tile([128, WP, HP], F16, tag="xA")
            xB = cp.tile([128, WP, HP], F16, tag="xB")
            x0l = cp.tile([128, WP, HP], F16, tag="x0l")
            x8A = cp.tile([128, 2, WP, H], F8, tag="x8A")
            x8B = cp.tile([128, 2, WP, H], F8, tag="x8B")
            wt8 = cp.tile([128, max(NPAIRS, 1), 2, 2, 128], F8, tag="wt8")
            wtb = cp.tile([128, NS, 2, 128], F16, tag="wtb")
            wf = cp.tile([128, 2, 64], BF16, tag="wf")
            bi = cp.tile([128, 2], F32, tag="bi")

            for c in range(4):
                r0, r1 = (WP * c) // 4, (WP * (c + 1)) // 4
                nc.sync.dma_start(xA[:, r0:r1, :], X0[:, r0:r1, :])
                nc.sync.dma_start(x0l[:, r0:r1, :], X0L[:, r0:r1, :])
                nc.sync.dma_start(x8A[:, :, r0:r1, :], X8[:, :, r0:r1, :])
            nc.sync.dma_start(wt8[:], WT8[:])
            nc.sync.dma_start(wtb[:], WTB[:])
            nc.sync.dma_start(wf[:], WF[:])
            nc.sync.dma_start(bi[:], BI[:])

            def dr_rhs(x8s, img, w0, t1, t2):
                """[64, 2@delta, 512@1] moving AP for a DoubleRow tap pair."""
                (a1, b1), (a2, b2) = t1, t2
                delta = (BIDX[b2] - BIDX[b1]) * CSZ + (a2 - a1) * H
                assert delta > 0 and delta % 16 == 0, (t1, t2, delta)
                base = x8s[img * 64:(img + 1) * 64, BIDX[b1],
                           w0 + a1:w0 + a1 + RPG, :]
                ap = [list(base.ap[0]), [delta, 2], [1, NPIX]]
                return AP(base.tensor, base.offset, ap)

            def emit_tail(p):
                """mm2 + gate + state update + fp8 copy refresh for a
                finished group (issued one group later: PE never stalls)."""
                hA, hB, gate, gateB, xs, xd, x8d, w0, t = p
                first, last = t == 0, t == STEPS - 1
                dx = pdxp.tile([128, NPIX], F32, tag="dx")
                for k in range(2):
                    nc.tensor.matmul(dx[0:64, :], wf[:, k, :], hA[:, k, :],
                                     start=k == 0, stop=k == 1,
                                     skip_group_check=True)
                    nc.tensor.matmul(dx[64:128, :], wf[:, k, :], hB[:, k, :],
                                     start=k == 0, stop=k == 1,
                                     skip_group_check=True,
                                     tile_position=(0, 64))
                tg = hp.tile([128, NPIX], F16, tag="tg")
                nc.vector.tensor_tensor(tg[0:64, :], dx[0:64, :],
                                        gate[0:64, :], ALU.mult)
                nc.vector.tensor_tensor(tg[64:128, :], dx[64:128, :],
                                        gateB[64:128, :], ALU.mult)
                tg3 = tg[:].rearrange("p (a b) -> p a b", a=RPG)
                rows = slice(w0 + 1, w0 + 1 + RPG)
                src = x0l if first else xs
                nc.vector.tensor_tensor(xd[:, rows, 1:1 + H], tg3,
                                        src[:, rows, 1:1 + H], ALU.add)
                # column halos (reflect) for this group's rows
                nc.vector.tensor_copy(xd[:, rows, 0], xd[:, rows, 2])
                nc.vector.tensor_copy(xd[:, rows, HP - 1], xd[:, rows, HP - 3])
                # packed fp8 b-copies (b=0 needs col halo 0; b=2 needs 129)
                nc.scalar.copy(x8d[:, 0, rows, :], xd[:, rows, 0:H])
                nc.vector.tensor_copy(x8d[:, 1, rows, :], xd[:, rows, 2:2 + H])
                if w0 == 0:
                    # row halos for next step's first group (needs rows 0..5)
                    nc.vector.tensor_copy(xd[:, 0, :], xd[:, 2, :])
                    nc.scalar.copy(x8d[:, 0, 0, :], xd[:, 0, 0:H])
                    nc.vector.tensor_copy(x8d[:, 1, 0, :], xd[:, 0, 2:2 + H])
                if w0 == W - RPG:
                    nc.vector.tensor_copy(xd[:, WP - 1, :], xd[:, WP - 3, :])
                    nc.scalar.copy(x8d[:, 0, WP - 1, :], xd[:, WP - 1, 0:H])
                    nc.vector.tensor_copy(x8d[:, 1, WP - 1, :],
                                          xd[:, WP - 1, 2:2 + H])
                if last:
                    nc.sync.dma_start(OUT[:, w0:w0 + RPG, :],
                                      xd[:, rows, 1:1 + H])

            pend = None
            for t in range(STEPS):
                xs, xd = (xA, xB) if t % 2 == 0 else (xB, xA)
                x8s, x8d = (x8A, x8B) if t % 2 == 0 else (x8B, x8A)
                for g in range(NG):
                    w0 = RPG * g

                    glA = gp.tile([1, NPIX], BF16, tag="glA")
                    glB = gp.tile([1, NPIX], BF16, tag="glB")
                    nc.sync.dma_start(glA[:], GL[t, g, 0:1, :])
                    nc.sync.dma_start(glB[:], GL[t, g, 1:2, :])
                    gate = gp.tile([128, NPIX], BF16, tag="gate")
                    gateB = gp.tile([128, NPIX], BF16, tag="gateB")
                    nc.gpsimd.partition_broadcast(gate[:, :], glA[:])
                    nc.gpsimd.partition_broadcast(gateB[:, :], glB[:])

                    phs = [[php.tile([128, NPIX], F32, tag=f"ph{im}{m}",
                                     name=f"ph{im}{m}")
                            for m in range(2)] for im in range(2)]
                    hA = hp.tile([128, 2, NPIX], BF16, tag="hA")
                    hB = hp.tile([128, 2, NPIX], BF16, tag="hB")
                    for m in range(2):
                        nmm = NPAIRS + NS
                        mi = 0
                        for pi, (t1, t2) in enumerate(PAIRS):
                            st, sp = mi == 0, mi == nmm - 1
                            for im in range(2):
                                nc.tensor.matmul(
                                    phs[im][m][:], wt8[im * 64:(im + 1) * 64, pi, m],
                                    dr_rhs(x8s, im, w0, t1, t2),
                                    start=st, stop=sp, perf_mode=DR,
                                    skip_group_check=True)
                            mi += 1
                        for si, (a, b) in enumerate(SINGLES):
                            st, sp = mi == 0, mi == nmm - 1
                            for im in range(2):
                                rhs = xs[im * 64:(im + 1) * 64,
                                         w0 + a:w0 + a + RPG, b:b + H]
                                nc.tensor.matmul(
                                    phs[im][m][:], wtb[im * 64:(im + 1) * 64, si, m],
                                    rhs, start=st, stop=sp,
                                    skip_group_check=True)
                            mi += 1
                        # relu + bias, PSUM -> SBUF bf16 (3 on ACT, 1 on DVE)
                        nc.scalar.activation(hA[:, m, :], phs[0][m][:], AF.Relu,
                                             bias=bi[:, m:m + 1])
                        nc.scalar.activation(hB[:, m, :], phs[1][m][:],
                                             AF.Relu, bias=bi[:, m:m + 1])
                        if m == 0 and pend is not None:
                            emit_tail(pend)
                            pend = None

                    if pend is not None:
                        emit_tail(pend)
                    pend = (hA, hB, gate, gateB, xs, xd, x8d, w0, t)

            emit_tail(pend)

    nc.compile()
    return nc


def _host_pack(x, w_conv1, w_conv2, w_hidden, b_hidden, w_final, rand_vals):
    import ml_dtypes
    bf16 = ml_dtypes.bfloat16
    f16 = np.float16
    e4m3 = ml_dtypes.float8_e4m3

    Wh = np.asarray(w_hidden, np.float64)            # [256, 192]
    w1 = np.asarray(w_conv1, np.float64)[:, 0]       # [64, 3, 3]
    w2 = np.asarray(w_conv2, np.float64)[:, 0]

    E = {}
    for a in range(3):
        for b in range(3):
            Et = Wh[:, 64:128] * w1[None, :, a, b] + Wh[:, 128:192] * w2[None, :, a, b]
            if (a, b) == (1, 1):
                Et = Et + Wh[:, 0:64]
            E[(a, b)] = Et                            # [256, 64]

    fp8taps = [tp for pr in PAIRS for tp in pr]
    if fp8taps:
        rowmax = np.max(np.stack([np.abs(E[tp]) for tp in fp8taps]), axis=(0, 2))
        s = np.clip(224.0 / np.maximum(rowmax, 1e-6), 0.25, 4096.0)   # [256]
    else:
        s = np.ones(HID)

    wt8 = np.zeros((128, max(NPAIRS, 1), 2, 2, 128), np.float32)
    for pi, (t1, t2) in enumerate(PAIRS):
        for ko, tp in enumerate((t1, t2)):
            Es = E[tp] * s[:, None]                   # [256, 64]
            for m in range(2):
                lhsT = Es[128 * m:128 * (m + 1), :].T                 # [64, 128]
                wt8[0:64, pi, m, ko, :] = lhsT
                wt8[64:128, pi, m, ko, :] = lhsT
    wt8 = wt8.astype(e4m3)

    wtb = np.zeros((128, NS, 2, 128), np.float32)
    for si, tp in enumerate(SINGLES):
        Es = E[tp] * s[:, None]
        for m in range(2):
            lhsT = Es[128 * m:128 * (m + 1), :].T
            wtb[0:64, si, m, :] = lhsT
            wtb[64:128, si, m, :] = lhsT
    wtb = wtb.astype(f16)

    bv = np.asarray(b_hidden, np.float64) * s
    bi = np.stack([bv[0:128], bv[128:256]], axis=1).astype(np.float32)

    wfz = np.asarray(w_final, np.float64).copy()     # [64, 256]
    wfz[0:4, :] = 0.0                                # immutable image channels
    wfT = (wfz / s[None, :]).T                       # [256, 64]
    wf = np.ascontiguousarray(
        np.stack([wfT[0:128], wfT[128:256]], axis=1)).astype(bf16)

    # life mask is static: channel-0 updates masked out -> life == (x0 > 0)
    Lhw = np.asarray(x)[..., 0] > 0                  # [B, H, W]
    Lwh = np.ascontiguousarray(Lhw.transpose(0, 2, 1))   # [B, W, H]
    G = np.asarray(rand_vals)[..., 0] > 0.5          # [S, B, H, W]
    GLw = G.transpose(0, 1, 3, 2) & Lwh[None]        # [S, B, W, H]

    x_chw = np.asarray(x, np.float32).transpose(0, 3, 2, 1)      # [B, C, W, H]
    xp = np.pad(x_chw, ((0, 0), (0, 0), (1, 1), (1, 1)), mode='reflect')
    xp = xp.astype(f16)
    xl = x_chw * Lwh[:, None].astype(np.float32)
    xpl = np.pad(xl, ((0, 0), (0, 0), (1, 1), (1, 1)),
                 mode='reflect').astype(f16)

    in_maps = []
    for i in range(NCORES):
        sl = slice(BPC * i, BPC * (i + 1))
        x0 = np.ascontiguousarray(xp[sl].reshape(BPC * C, WP, HP))
        x0l = np.ascontiguousarray(xpl[sl].reshape(BPC * C, WP, HP))
        x8 = np.stack([x0[:, :, 0:H], x0[:, :, 2:2 + H]], axis=1)
        x8 = np.ascontiguousarray(x8).astype(e4m3)
        glc = np.ascontiguousarray(
            GLw[:, sl].reshape(STEPS, BPC, NG, NPIX).transpose(0, 2, 1, 3)
        ).astype(bf16)
        in_maps.append({
            "x0": x0, "x0l": x0l, "x8": x8,
            "wt8": wt8, "wtb": wtb, "wf": wf, "bi": bi, "gl": glc,
        })
    return in_maps


def _run(inputs, trace=False, trace_kwargs=None):
    from concourse.bass_utils import run_bass_kernel_spmd
    if "nc" not in _nc_cache:
        _nc_cache["nc"] = _build()
    nc = _nc_cache["nc"]
    in_maps = _host_pack(
        inputs["x"], inputs["w_conv1"], inputs["w_conv2"], inputs["w_hidden"],
        inputs["b_hidden"], inputs["w_final"], inputs["rand_vals"])
    kwargs = {}
    if trace:
        kwargs["trace"] = True
        if trace_kwargs:
            kwargs.update(trace_kwargs)
    res = run_bass_kernel_spmd(nc, in_maps, core_ids=list(range(NCORES)), **kwargs)
    outs = []
    for i in range(NCORES):
        o = res.results[i]["out"].astype(np.float32).reshape(BPC, C, W, H)
        outs.append(o.transpose(0, 3, 2, 1))         # -> [b, H, W, C]
    full = np.concatenate(outs, axis=0).astype(np.float32)
    return full, res


def kernel(**inputs) -> np.ndarray:
    steps = int(np.asarray(inputs.get("steps", STEPS)))
    assert steps == STEPS, f"kernel compiled for {STEPS} steps, got {steps}"
    out, _ = _run(inputs)
    return out


# revision 10
# speedup vs baseline: 4.1853x; 1.3221x over previous
"""Trainium2 Bass kernel for nn_BasicNCAModel (neural cellular automaton).

Strategy (pure data parallelism, batch 16 -> 2 images per core x 8 cores):

* State layout per core: [128 partitions = 2 images x 64 channels,
  130 x 130 reflect-padded grid] in SBUF fp16, ping-pong buffered.
* The two depthwise 3x3 convs are folded into the hidden matmul:
  h = relu(sum_tap E_tap @ x_shift(tap) + b) with E_tap[256, 64].
* fp8 DoubleRow: NPAIRS of the 9 taps run as e4m3 DoubleRow matmuls that
  contract TWO taps per instruction (2 fp8 weights/PE cell).  The moving
  operand comes from packed fp8 state copies x8[b-idx][130 rows, 128 cols]
  (H-stride 128, no column halo) so a group's 512 pixels are contiguous;
  the K-pair stride (delta = d_bidx*16640 + d_a*128) is 16B-aligned as
  DoubleRow requires.  Remaining taps (incl. the W0-carrying center) stay
  fp16 for accuracy; per-hidden-row scales (folded into bias and w_final)
  keep the e4m3 weights in range.
* No per-step barrier: halo cols are refreshed per group, halo rows right
  after the first/last group of each step, so consecutive steps pipeline
  on the PE without HAM re-throttle.
* Stochastic fire gate (pre-merged with the static life mask on the host)
  is broadcast per group on GpSimd and applied on DVE; the masked initial
  state x0*L feeds the first update so step 0 needs no life tiles.
"""
import sys
sys.path.insert(0, '/opt/trn_rl_repo')

import numpy as np

B, H, W, C = 16, 128, 128, 64
HID = 256
STEPS = 8
NCORES = 8
BPC = B // NCORES            # images per core = 2
WP, HP = W + 2, H + 2        # padded grid 130 x 130
RPG = 4                      # W-rows per group
NPIX = RPG * H               # 512 pixels per matmul tile
NG = W // RPG                # 32 groups per step
CSZ = WP * H                 # 16640 elements per packed fp8 copy

NPAIRS = 3                   # fp8 DoubleRow tap pairs (0, 2 or 3)

# tap schedule: pairs are e4m3 DoubleRow (2 taps/MM); singles + center fp16.
# b-copies: bidx 0 <-> b=0 (cols 0:128 of padded state), 1 <-> b=2 (cols 2:130)
BIDX = {0: 0, 2: 1}
if NPAIRS == 3:
    PAIRS = [((0, 0), (1, 0)), ((2, 0), (0, 2)), ((1, 2), (2, 2))]
    SINGLES = [(0, 1), (2, 1), (1, 1)]
elif NPAIRS == 2:
    PAIRS = [((0, 0), (1, 0)), ((1, 2), (2, 2))]
    SINGLES = [(2, 0), (0, 2), (0, 1), (2, 1), (1, 1)]
else:
    PAIRS = []
    SINGLES = [(a, b) for a in range(3) for b in range(3)]
NS = len(SINGLES)

_nc_cache = {}


def _build():
    import concourse.bacc as bacc
    import concourse.mybir as mybir
    import concourse.tile as tile
    from concourse.bass import AP

    F32 = mybir.dt.float32
    F16 = mybir.dt.float16
    BF16 = mybir.dt.bfloat16
    F8 = mybir.dt.float8e4
    AF = mybir.ActivationFunctionType
    ALU = mybir.AluOpType
    DR = mybir.MatmulPerfMode.DoubleRow

    nc = bacc.Bacc("TRN2", target_bir_lowering=False, debug=False,
                   enable_asserts=False, num_devices=NCORES)

    X0 = nc.dram_tensor("x0", [128, WP, HP], F16, kind="ExternalInput")
    X0L = nc.dram_tensor("x0l", [128, WP, HP], F16, kind="ExternalInput")
    X8 = nc.dram_tensor("x8", [128, 2, WP, H], F8, kind="ExternalInput")
    WT8 = nc.dram_tensor("wt8", [128, max(NPAIRS, 1), 2, 2, 128], F8,
                         kind="ExternalInput")
    WTB = nc.dram_tensor("wtb", [128, NS, 2, 128], F16, kind="ExternalInput")
    WF = nc.dram_tensor("wf", [128, 2, 64], BF16, kind="ExternalInput")
    BI = nc.dram_tensor("bi", [128, 2], F32, kind="ExternalInput")
    GL = nc.dram_tensor("gl", [STEPS, NG, 2, NPIX], BF16, kind="ExternalInput")
    OUT = nc.dram_tensor("out", [128, W, H], F16, kind="ExternalOutput")

    with tile.TileContext(nc) as tc:
        with tc.tile_pool(name="const", bufs=1) as cp, \
             tc.tile_pool(name="hbuf", bufs=2) as hp, \
             tc.tile_pool(name="gbuf", bufs=3) as gp, \
             tc.tile_pool(name="ph", bufs=1, space="PSUM") as php, \
             tc.tile_pool(name="pdx", bufs=2, space="PSUM") as pdxp:

            xA = cp.tile([128, WP, HP], F16, tag="xA")
            xB = cp.tile([128, WP, HP], F16, tag="xB")
            x0l = cp.tile([128, WP, HP], F16, tag="x0l")
            x8A = cp.tile([128, 2, WP, H], F8, tag="x8A")
            x8B = cp.tile([128, 2, WP, H], F8, tag="x8B")
            wt8 = cp.tile([128, max(NPAIRS, 1), 2, 2, 128], F8, tag="wt8")
            wtb = cp.tile([128, NS, 2, 128], F16, tag="wtb")
            wf = cp.tile([128, 2, 64], BF16, tag="wf")
            bi = cp.tile([128, 2], F32, tag="bi")

            for c in range(4):
                r0, r1 = (WP * c) // 4, (WP * (c + 1)) // 4
                nc.sync.dma_start(xA[:, r0:r1, :], X0[:, r0:r1, :])
                nc.sync.dma_start(x0l[:, r0:r1, :], X0L[:, r0:r1, :])
                nc.sync.dma_start(x8A[:, :, r0:r1, :], X8[:, :, r0:r1, :])
            nc.sync.dma_start(wt8[:], WT8[:])
            nc.sync.dma_start(wtb[:], WTB[:])
            nc.sync.dma_start(wf[:], WF[:])
            nc.sync.dma_start(bi[:], BI[:])

            def dr_rhs(x8s, img, w0, t1, t2):
                """[64, 2@delta, 512@1] moving AP for a DoubleRow tap pair."""
                (a1, b1), (a2, b2) = t1, t2
                delta = (BIDX[b2] - BIDX[b1]) * CSZ + (a2 - a1) * H
                assert delta > 0 and delta % 16 == 0, (t1, t2, delta)
                base = x8s[img * 64:(img + 1) * 64, BIDX[b1],
                           w0 + a1:w0 + a1 + RPG, :]
                ap = [list(base.ap[0]), [delta, 2], [1, NPIX]]
                return AP(base.tensor, base.offset, ap)

            def emit_tail(p):
                """mm2 + gate + state update + fp8 copy refresh for a
                finished group (issued one group later: PE never stalls)."""
                hA, hB, gate, gateB, xs, xd, x8d, w0, t = p
                first, last = t == 0, t == STEPS - 1
                dx = pdxp.tile([128, NPIX], F32, tag="dx")
                for k in range(2):
                    nc.tensor.matmul(dx[0:64, :], wf[:, k, :], hA[:, k, :],
                                     start=k == 0, stop=k == 1,
                                     skip_group_check=True)
                    nc.tensor.matmul(dx[64:128, :], wf[:, k, :], hB[:, k, :],
                                     start=k == 0, stop=k == 1,
                                     skip_group_check=True,
                                     tile_position=(0, 64))
                tg = hp.tile([128, NPIX], F16, tag="tg")
                nc.vector.tensor_tensor(tg[0:64, :], dx[0:64, :],
                                        gate[0:64, :], ALU.mult)
                nc.vector.tensor_tensor(tg[64:128, :], dx[64:128, :],
                                        gateB[64:128, :], ALU.mult)
                tg3 = tg[:].rearrange("p (a b) -> p a b", a=RPG)
                rows = slice(w0 + 1, w0 + 1 + RPG)
                src = x0l if first else xs
                nc.vector.tensor_tensor(xd[:, rows, 1:1 + H], tg3,
                                        src[:, rows, 1:1 + H], ALU.add)
                # packed fp8 b-copies; reflect halo columns read directly
                # (interior cols 1..128 only: nothing reads state cols 0/129)
                nc.scalar.copy(x8d[:, 0, rows, 1:H], xd[:, rows, 1:H])
                nc.scalar.copy(x8d[:, 0, rows, 0:1], xd[:, rows, 2:3])
                nc.vector.tensor_copy(x8d[:, 1, rows, 0:H - 1],
                                      xd[:, rows, 2:1 + H])
                nc.vector.tensor_copy(x8d[:, 1, rows, H - 1:H],
                                      xd[:, rows, H - 1:H])
                if w0 == 0:
                    # reflect row halos for next step's first group
                    nc.vector.tensor_copy(xd[:, 0, 1:1 + H], xd[:, 2, 1:1 + H])
                    nc.scalar.copy(x8d[:, 0, 0, 1:H], xd[:, 2, 1:H])
                    nc.scalar.copy(x8d[:, 0, 0, 0:1], xd[:, 2, 2:3])
                    nc.vector.tensor_copy(x8d[:, 1, 0, 0:H - 1], xd[:, 2, 2:1 + H])
                    nc.vector.tensor_copy(x8d[:, 1, 0, H - 1:H],
                                          xd[:, 2, H - 1:H])
                if w0 == W - RPG:
                    r = WP - 3
                    nc.vector.tensor_copy(xd[:, WP - 1, 1:1 + H], xd[:, r, 1:1 + H])
                    nc.scalar.copy(x8d[:, 0, WP - 1, 1:H], xd[:, r, 1:H])
                    nc.scalar.copy(x8d[:, 0, WP - 1, 0:1], xd[:, r, 2:3])
                    nc.vector.tensor_copy(x8d[:, 1, WP - 1, 0:H - 1],
                                          xd[:, r, 2:1 + H])
                    nc.vector.tensor_copy(x8d[:, 1, WP - 1, H - 1:H],
                                          xd[:, r, H - 1:H])
                if last:
                    nc.sync.dma_start(OUT[:, w0:w0 + RPG, :],
                                      xd[:, rows, 1:1 + H])

            pend = None
            for t in range(STEPS):
                xs, xd = (xA, xB) if t % 2 == 0 else (xB, xA)
                x8s, x8d = (x8A, x8B) if t % 2 == 0 else (x8B, x8A)
                for g in range(NG):
                    w0 = RPG * g

                    glA = gp.tile([1, NPIX], BF16, tag="glA")
                    glB = gp.tile([1, NPIX], BF16, tag="glB")
                    nc.sync.dma_start(glA[:], GL[t, g, 0:1, :])
                    nc.sync.dma_start(glB[:], GL[t, g, 1:2, :])
                    gate = gp.tile([128, NPIX], BF16, tag="gate")
                    gateB = gp.tile([128, NPIX], BF16, tag="gateB")
                    nc.gpsimd.partition_broadcast(gate[:, :], glA[:])
                    nc.gpsimd.partition_broadcast(gateB[:, :], glB[:])

                    phs = [[php.tile([128, NPIX], F32, tag=f"ph{im}{m}",
                                     name=f"ph{im}{m}")
                            for m in range(2)] for im in range(2)]
                    hA = hp.tile([128, 2, NPIX], BF16, tag="hA")
                    hB = hp.tile([128, 2, NPIX], BF16, tag="hB")
                    for m in range(2):
                        nmm = NPAIRS + NS
                        mi = 0
                        for pi, (t1, t2) in enumerate(PAIRS):
                            st, sp = mi == 0, mi == nmm - 1
                            for im in range(2):
                                nc.tensor.matmul(
                                    phs[im][m][:], wt8[im * 64:(im + 1) * 64, pi, m],
                                    dr_rhs(x8s, im, w0, t1, t2),
                                    start=st, stop=sp, perf_mode=DR,
                                    skip_group_check=True)
                            mi += 1
                        for si, (a, b) in enumerate(SINGLES):
                            st, sp = mi == 0, mi == nmm - 1
                            for im in range(2):
                                rhs = xs[im * 64:(im + 1) * 64,
                                         w0 + a:w0 + a + RPG, b:b + H]
                                nc.tensor.matmul(
                                    phs[im][m][:], wtb[im * 64:(im + 1) * 64, si, m],
                                    rhs, start=st, stop=sp,
                                    skip_group_check=True)
                            mi += 1
                        # relu + bias, PSUM -> SBUF bf16 (3 on ACT, 1 on DVE)
                        nc.scalar.activation(hA[:, m, :], phs[0][m][:], AF.Relu,
                                             bias=bi[:, m:m + 1])
                        nc.scalar.activation(hB[:, m, :], phs[1][m][:],
                                             AF.Relu, bias=bi[:, m:m + 1])

                    if pend is not None:
                        emit_tail(pend)
                    pend = (hA, hB, gate, gateB, xs, xd, x8d, w0, t)

            emit_tail(pend)

    nc.compile()
    return nc


def _host_pack(x, w_conv1, w_conv2, w_hidden, b_hidden, w_final, rand_vals):
    import ml_dtypes
    bf16 = ml_dtypes.bfloat16
    f16 = np.float16
    e4m3 = ml_dtypes.float8_e4m3

    Wh = np.asarray(w_hidden, np.float64)            # [256, 192]
    w1 = np.asarray(w_conv1, np.float64)[:, 0]       # [64, 3, 3]
    w2 = np.asarray(w_conv2, np.float64)[:, 0]

    E = {}
    for a in range(3):
        for b in range(3):
            Et = Wh[:, 64:128] * w1[None, :, a, b] + Wh[:, 128:192] * w2[None, :, a, b]
            if (a, b) == (1, 1):
                Et = Et + Wh[:, 0:64]
            E[(a, b)] = Et                            # [256, 64]

    fp8taps = [tp for pr in PAIRS for tp in pr]
    if fp8taps:
        rowmax = np.max(np.stack([np.abs(E[tp]) for tp in fp8taps]), axis=(0, 2))
        s = np.clip(224.0 / np.maximum(rowmax, 1e-6), 0.25, 4096.0)   # [256]
    else:
        s = np.ones(HID)

    wt8 = np.zeros((128, max(NPAIRS, 1), 2, 2, 128), np.float32)
    for pi, (t1, t2) in enumerate(PAIRS):
        for ko, tp in enumerate((t1, t2)):
            Es = E[tp] * s[:, None]                   # [256, 64]
            for m in range(2):
                lhsT = Es[128 * m:128 * (m + 1), :].T                 # [64, 128]
                wt8[0:64, pi, m, ko, :] = lhsT
                wt8[64:128, pi, m, ko, :] = lhsT
    wt8 = wt8.astype(e4m3)

    wtb = np.zeros((128, NS, 2, 128), np.float32)
    for si, tp in enumerate(SINGLES):
        Es = E[tp] * s[:, None]
        for m in range(2):
            lhsT = Es[128 * m:128 * (m + 1), :].T
            wtb[0:64, si, m, :] = lhsT
            wtb[64:128, si, m, :] = lhsT
    wtb = wtb.astype(f16)

    bv = np.asarray(b_hidden, np.float64) * s
    bi = np.stack([bv[0:128], bv[128:256]], axis=1).astype(np.float32)

    wfz = np.asarray(w_final, np.float64).copy()     # [64, 256]
    wfz[0:4, :] = 0.0                                # immutable image channels
    wfT = (wfz / s[None, :]).T                       # [256, 64]
    wf = np.ascontiguousarray(
        np.stack([wfT[0:128], wfT[128:256]], axis=1)).astype(bf16)

    # life mask is static: channel-0 updates masked out -> life == (x0 > 0)
    Lhw = np.asarray(x)[..., 0] > 0                  # [B, H, W]
    Lwh = np.ascontiguousarray(Lhw.transpose(0, 2, 1))   # [B, W, H]
    G = np.asarray(rand_vals)[..., 0] > 0.5          # [S, B, H, W]
    GLw = G.transpose(0, 1, 3, 2) & Lwh[None]        # [S, B, W, H]

    x_chw = np.asarray(x, np.float32).transpose(0, 3, 2, 1)      # [B, C, W, H]
    xp = np.pad(x_chw, ((0, 0), (0, 0), (1, 1), (1, 1)), mode='reflect')
    xp = xp.astype(f16)
    xl = x_chw * Lwh[:, None].astype(np.float32)
    xpl = np.pad(xl, ((0, 0), (0, 0), (1, 1), (1, 1)),
                 mode='reflect').astype(f16)

    in_maps = []
    for i in range(NCORES):
        sl = slice(BPC * i, BPC * (i + 1))
        x0 = np.ascontiguousarray(xp[sl].reshape(BPC * C, WP, HP))
        x0l = np.ascontiguousarray(xpl[sl].reshape(BPC * C, WP, HP))
        x8 = np.stack([x0[:, :, 0:H], x0[:, :, 2:2 + H]], axis=1)
        x8 = np.ascontiguousarray(x8).astype(e4m3)
        glc = np.ascontiguousarray(
            GLw[:, sl].reshape(STEPS, BPC, NG, NPIX).transpose(0, 2, 1, 3)
        ).astype(bf16)
        in_maps.append({
            "x0": x0, "x0l": x0l, "x8": x8,
            "wt8": wt8, "wtb": wtb, "wf": wf, "bi": bi, "gl": glc,
        })
    return in_maps


def _run(inputs, trace=False, trace_kwargs=None):
    from concourse.bass_utils import run_bass_kernel_spmd
    if "nc" not in _nc_cache:
        _nc_cache["nc"] = _build()
    nc = _nc_cache["nc"]
    in_maps = _host_pack(
        inputs["x"], inputs["w_conv1"], inputs["w_conv2"], inputs["w_hidden"],
        inputs["b_hidden"], inputs["w_final"], inputs["rand_vals"])
    kwargs = {}
    if trace:
        kwargs["trace"] = True
        if trace_kwargs:
            kwargs.update(trace_kwargs)
    res = run_bass_kernel_spmd(nc, in_maps, core_ids=list(range(NCORES)), **kwargs)
    outs = []
    for i in range(NCORES):
        o = res.results[i]["out"].astype(np.float32).reshape(BPC, C, W, H)
        outs.append(o.transpose(0, 3, 2, 1))         # -> [b, H, W, C]
    full = np.concatenate(outs, axis=0).astype(np.float32)
    return full, res


def kernel(**inputs) -> np.ndarray:
    steps = int(np.asarray(inputs.get("steps", STEPS)))
    assert steps == STEPS, f"kernel compiled for {STEPS} steps, got {steps}"
    out, _ = _run(inputs)
    return out


# revision 16
# speedup vs baseline: 4.3638x; 1.0426x over previous
"""Trainium2 Bass kernel for nn_BasicNCAModel (neural cellular automaton).

Strategy (pure data parallelism, batch 16 -> 2 images per core x 8 cores):

* State layout per core: [128 partitions = 2 images x 64 channels,
  130 x 130 reflect-padded grid] in SBUF fp16, ping-pong buffered.
* The two depthwise 3x3 convs are folded into the hidden matmul:
  h = relu(sum_tap E_tap @ x_shift(tap) + b) with E_tap[256, 64].
* fp8 DoubleRow: NPAIRS of the 9 taps run as e4m3 DoubleRow matmuls that
  contract TWO taps per instruction (2 fp8 weights/PE cell).  The moving
  operand comes from packed fp8 state copies x8[b-idx][130 rows, 128 cols]
  (H-stride 128, no column halo) so a group's 512 pixels are contiguous;
  the K-pair stride (delta = d_bidx*16640 + d_a*128) is 16B-aligned as
  DoubleRow requires.  Remaining taps (incl. the W0-carrying center) stay
  fp16 for accuracy; per-hidden-row scales (folded into bias and w_final)
  keep the e4m3 weights in range.
* No per-step barrier: halo cols are refreshed per group, halo rows right
  after the first/last group of each step, so consecutive steps pipeline
  on the PE without HAM re-throttle.
* Stochastic fire gate (pre-merged with the static life mask on the host)
  is broadcast per group on GpSimd and applied on DVE; the masked initial
  state x0*L feeds the first update so step 0 needs no life tiles.
"""
import sys
sys.path.insert(0, '/opt/trn_rl_repo')

import numpy as np

B, H, W, C = 16, 128, 128, 64
HID = 256
STEPS = 8
NCORES = 8
BPC = B // NCORES            # images per core = 2
WP, HP = W + 2, H + 2        # padded grid 130 x 130
RPG = 4                      # W-rows per group
NPIX = RPG * H               # 512 pixels per matmul tile
NG = W // RPG                # 32 groups per step
CSZ = WP * H                 # 16640 elements per packed fp8 copy

NPAIRS = 3                   # fp8 DoubleRow tap pairs (0, 2 or 3)

# tap schedule: pairs are e4m3 DoubleRow (2 taps/MM); singles + center fp16.
# b-copies: bidx 0 <-> b=0 (cols 0:128 of padded state), 1 <-> b=2 (cols 2:130)
BIDX = {0: 0, 2: 1}
if NPAIRS == 3:
    PAIRS = [((0, 0), (1, 0)), ((2, 0), (0, 2)), ((1, 2), (2, 2))]
    SINGLES = [(0, 1), (2, 1), (1, 1)]
elif NPAIRS == 2:
    PAIRS = [((0, 0), (1, 0)), ((1, 2), (2, 2))]
    SINGLES = [(2, 0), (0, 2), (0, 1), (2, 1), (1, 1)]
else:
    PAIRS = []
    SINGLES = [(a, b) for a in range(3) for b in range(3)]
NS = len(SINGLES)

_nc_cache = {}


def _build():
    import concourse.bacc as bacc
    import concourse.mybir as mybir
    import concourse.tile as tile
    from concourse.bass import AP

    F32 = mybir.dt.float32
    F16 = mybir.dt.float16
    BF16 = mybir.dt.bfloat16
    F8 = mybir.dt.float8e4
    AF = mybir.ActivationFunctionType
    ALU = mybir.AluOpType
    DR = mybir.MatmulPerfMode.DoubleRow

    nc = bacc.Bacc("TRN2", target_bir_lowering=False, debug=False,
                   enable_asserts=False, num_devices=NCORES)

    X0 = nc.dram_tensor("x0", [128, WP, HP], F16, kind="ExternalInput")
    X0L = nc.dram_tensor("x0l", [128, WP, HP], F16, kind="ExternalInput")
    X8 = nc.dram_tensor("x8", [128, 2, WP, H], F8, kind="ExternalInput")
    WT8 = nc.dram_tensor("wt8", [128, max(NPAIRS, 1), 2, 2, 128], F8,
                         kind="ExternalInput")
    WTB = nc.dram_tensor("wtb", [128, NS, 2, 128], F16, kind="ExternalInput")
    WF = nc.dram_tensor("wf", [128, 2, 64], BF16, kind="ExternalInput")
    BI = nc.dram_tensor("bi", [128, 2], F32, kind="ExternalInput")
    GL = nc.dram_tensor("gl", [STEPS, NG, 128, NPIX], BF16, kind="ExternalInput")
    OUT = nc.dram_tensor("out", [128, W, H], F16, kind="ExternalOutput")

    with tile.TileContext(nc) as tc:
        with tc.tile_pool(name="const", bufs=1) as cp, \
             tc.tile_pool(name="hbuf", bufs=2) as hp, \
             tc.tile_pool(name="gbuf", bufs=3) as gp, \
             tc.tile_pool(name="ph", bufs=1, space="PSUM") as php, \
             tc.tile_pool(name="pdx", bufs=2, space="PSUM") as pdxp:

            xA = cp.tile([128, WP, HP], F16, tag="xA")
            xB = cp.tile([128, WP, HP], F16, tag="xB")
            x0l = cp.tile([128, WP, HP], F16, tag="x0l")
            x8A = cp.tile([128, 2, WP, H], F8, tag="x8A")
            x8B = cp.tile([128, 2, WP, H], F8, tag="x8B")
            wt8 = cp.tile([128, max(NPAIRS, 1), 2, 2, 128], F8, tag="wt8")
            wtb = cp.tile([128, NS, 2, 128], F16, tag="wtb")
            wf = cp.tile([128, 2, 64], BF16, tag="wf")
            bi = cp.tile([128, 2], F32, tag="bi")

            for c in range(4):
                r0, r1 = (WP * c) // 4, (WP * (c + 1)) // 4
                nc.sync.dma_start(xA[:, r0:r1, :], X0[:, r0:r1, :])
                nc.sync.dma_start(x0l[:, r0:r1, :], X0L[:, r0:r1, :])
                nc.sync.dma_start(x8A[:, :, r0:r1, :], X8[:, :, r0:r1, :])
            nc.sync.dma_start(wt8[:], WT8[:])
            nc.sync.dma_start(wtb[:], WTB[:])
            nc.sync.dma_start(wf[:], WF[:])
            nc.sync.dma_start(bi[:], BI[:])

            def dr_rhs(x8s, img, w0, t1, t2):
                """[64, 2@delta, 512@1] moving AP for a DoubleRow tap pair."""
                (a1, b1), (a2, b2) = t1, t2
                delta = (BIDX[b2] - BIDX[b1]) * CSZ + (a2 - a1) * H
                assert delta > 0 and delta % 16 == 0, (t1, t2, delta)
                base = x8s[img * 64:(img + 1) * 64, BIDX[b1],
                           w0 + a1:w0 + a1 + RPG, :]
                ap = [list(base.ap[0]), [delta, 2], [1, NPIX]]
                return AP(base.tensor, base.offset, ap)

            def emit_tail(p):
                """mm2 + gate + state update + fp8 copy refresh for a
                finished group (issued one group later: PE never stalls)."""
                hA, hB, gate, gateB, xs, xd, x8d, w0, t = p
                first, last = t == 0, t == STEPS - 1
                dx = pdxp.tile([128, NPIX], F32, tag="dx")
                for k in range(2):
                    nc.tensor.matmul(dx[0:64, :], wf[:, k, :], hA[:, k, :],
                                     start=k == 0, stop=k == 1,
                                     skip_group_check=True)
                    nc.tensor.matmul(dx[64:128, :], wf[:, k, :], hB[:, k, :],
                                     start=k == 0, stop=k == 1,
                                     skip_group_check=True,
                                     tile_position=(0, 64))
                tg = hp.tile([128, NPIX], F16, tag="tg")
                nc.vector.tensor_tensor(tg[:], dx[:], gate[:], ALU.mult)
                tg3 = tg[:].rearrange("p (a b) -> p a b", a=RPG)
                rows = slice(w0 + 1, w0 + 1 + RPG)
                src = x0l if first else xs
                nc.vector.tensor_tensor(xd[:, rows, 1:1 + H], tg3,
                                        src[:, rows, 1:1 + H], ALU.add)
                # packed fp8 b-copies; reflect halo columns read directly
                # (interior cols 1..128 only: nothing reads state cols 0/129)
                nc.scalar.copy(x8d[:, 0, rows, 1:H], xd[:, rows, 1:H])
                nc.scalar.copy(x8d[:, 0, rows, 0:1], xd[:, rows, 2:3])
                nc.vector.tensor_copy(x8d[:, 1, rows, 0:H - 1],
                                      xd[:, rows, 2:1 + H])
                nc.vector.tensor_copy(x8d[:, 1, rows, H - 1:H],
                                      xd[:, rows, H - 1:H])
                if w0 == 0:
                    # reflect row halos for next step's first group
                    nc.vector.tensor_copy(xd[:, 0, 1:1 + H], xd[:, 2, 1:1 + H])
                    nc.scalar.copy(x8d[:, 0, 0, 1:H], xd[:, 2, 1:H])
                    nc.scalar.copy(x8d[:, 0, 0, 0:1], xd[:, 2, 2:3])
                    nc.vector.tensor_copy(x8d[:, 1, 0, 0:H - 1], xd[:, 2, 2:1 + H])
                    nc.vector.tensor_copy(x8d[:, 1, 0, H - 1:H],
                                          xd[:, 2, H - 1:H])
                if w0 == W - RPG:
                    r = WP - 3
                    nc.vector.tensor_copy(xd[:, WP - 1, 1:1 + H], xd[:, r, 1:1 + H])
                    nc.scalar.copy(x8d[:, 0, WP - 1, 1:H], xd[:, r, 1:H])
                    nc.scalar.copy(x8d[:, 0, WP - 1, 0:1], xd[:, r, 2:3])
                    nc.vector.tensor_copy(x8d[:, 1, WP - 1, 0:H - 1],
                                          xd[:, r, 2:1 + H])
                    nc.vector.tensor_copy(x8d[:, 1, WP - 1, H - 1:H],
                                          xd[:, r, H - 1:H])
                if last:
                    nc.sync.dma_start(OUT[:, w0:w0 + RPG, :],
                                      xd[:, rows, 1:1 + H])

            pend = None
            for t in range(STEPS):
                xs, xd = (xA, xB) if t % 2 == 0 else (xB, xA)
                x8s, x8d = (x8A, x8B) if t % 2 == 0 else (x8B, x8A)
                for g in range(NG):
                    w0 = RPG * g

                    gate = gp.tile([128, NPIX], BF16, tag="gate")
                    gateB = gate
                    nc.sync.dma_start(gate[:], GL[t, g])

                    phs = [[php.tile([128, NPIX], F32, tag=f"ph{im}{m}",
                                     name=f"ph{im}{m}")
                            for m in range(2)] for im in range(2)]
                    hA = hp.tile([128, 2, NPIX], BF16, tag="hA")
                    hB = hp.tile([128, 2, NPIX], BF16, tag="hB")
                    for m in range(2):
                        nmm = NPAIRS + NS
                        mi = 0
                        for pi, (t1, t2) in enumerate(PAIRS):
                            st, sp = mi == 0, mi == nmm - 1
                            for im in range(2):
                                nc.tensor.matmul(
                                    phs[im][m][:], wt8[im * 64:(im + 1) * 64, pi, m],
                                    dr_rhs(x8s, im, w0, t1, t2),
                                    start=st, stop=sp, perf_mode=DR,
                                    skip_group_check=True)
                            mi += 1
                        for si, (a, b) in enumerate(SINGLES):
                            st, sp = mi == 0, mi == nmm - 1
                            for im in range(2):
                                rhs = xs[im * 64:(im + 1) * 64,
                                         w0 + a:w0 + a + RPG, b:b + H]
                                nc.tensor.matmul(
                                    phs[im][m][:], wtb[im * 64:(im + 1) * 64, si, m],
                                    rhs, start=st, stop=sp,
                                    skip_group_check=True)
                            mi += 1
                        # relu + bias, PSUM -> SBUF bf16 (3 on ACT, 1 on DVE)
                        nc.scalar.activation(hA[:, m, :], phs[0][m][:], AF.Relu,
                                             bias=bi[:, m:m + 1])
                        if m == 0:
                            nc.scalar.activation(hB[:, m, :], phs[1][m][:],
                                                 AF.Relu, bias=bi[:, m:m + 1])
                        else:
                            nc.vector.tensor_scalar(
                                out=hB[:, m, :], in0=phs[1][m][:],
                                scalar1=bi[:, m:m + 1], scalar2=0.0,
                                op0=ALU.add, op1=ALU.max)

                    if pend is not None:
                        emit_tail(pend)
                    pend = (hA, hB, gate, gateB, xs, xd, x8d, w0, t)

            emit_tail(pend)

    nc.compile()
    return nc


def _host_pack(x, w_conv1, w_conv2, w_hidden, b_hidden, w_final, rand_vals):
    import ml_dtypes
    bf16 = ml_dtypes.bfloat16
    f16 = np.float16
    e4m3 = ml_dtypes.float8_e4m3

    Wh = np.asarray(w_hidden, np.float64)            # [256, 192]
    w1 = np.asarray(w_conv1, np.float64)[:, 0]       # [64, 3, 3]
    w2 = np.asarray(w_conv2, np.float64)[:, 0]

    E = {}
    for a in range(3):
        for b in range(3):
            Et = Wh[:, 64:128] * w1[None, :, a, b] + Wh[:, 128:192] * w2[None, :, a, b]
            if (a, b) == (1, 1):
                Et = Et + Wh[:, 0:64]
            E[(a, b)] = Et                            # [256, 64]

    fp8taps = [tp for pr in PAIRS for tp in pr]
    if fp8taps:
        rowmax = np.max(np.stack([np.abs(E[tp]) for tp in fp8taps]), axis=(0, 2))
        s = np.clip(224.0 / np.maximum(rowmax, 1e-6), 0.25, 4096.0)   # [256]
    else:
        s = np.ones(HID)

    wt8 = np.zeros((128, max(NPAIRS, 1), 2, 2, 128), np.float32)
    for pi, (t1, t2) in enumerate(PAIRS):
        for ko, tp in enumerate((t1, t2)):
            Es = E[tp] * s[:, None]                   # [256, 64]
            for m in range(2):
                lhsT = Es[128 * m:128 * (m + 1), :].T                 # [64, 128]
                wt8[0:64, pi, m, ko, :] = lhsT
                wt8[64:128, pi, m, ko, :] = lhsT
    wt8 = wt8.astype(e4m3)

    wtb = np.zeros((128, NS, 2, 128), np.float32)
    for si, tp in enumerate(SINGLES):
        Es = E[tp] * s[:, None]
        for m in range(2):
            lhsT = Es[128 * m:128 * (m + 1), :].T
            wtb[0:64, si, m, :] = lhsT
            wtb[64:128, si, m, :] = lhsT
    wtb = wtb.astype(f16)

    bv = np.asarray(b_hidden, np.float64) * s
    bi = np.stack([bv[0:128], bv[128:256]], axis=1).astype(np.float32)

    wfz = np.asarray(w_final, np.float64).copy()     # [64, 256]
    wfz[0:4, :] = 0.0                                # immutable image channels
    wfT = (wfz / s[None, :]).T                       # [256, 64]
    wf = np.ascontiguousarray(
        np.stack([wfT[0:128], wfT[128:256]], axis=1)).astype(bf16)

    # life mask is static: channel-0 updates masked out -> life == (x0 > 0)
    Lhw = np.asarray(x)[..., 0] > 0                  # [B, H, W]
    Lwh = np.ascontiguousarray(Lhw.transpose(0, 2, 1))   # [B, W, H]
    G = np.asarray(rand_vals)[..., 0] > 0.5          # [S, B, H, W]
    GLw = G.transpose(0, 1, 3, 2) & Lwh[None]        # [S, B, W, H]

    x_chw = np.asarray(x, np.float32).transpose(0, 3, 2, 1)      # [B, C, W, H]
    xp = np.pad(x_chw, ((0, 0), (0, 0), (1, 1), (1, 1)), mode='reflect')
    xp = xp.astype(f16)
    xl = x_chw * Lwh[:, None].astype(np.float32)
    xpl = np.pad(xl, ((0, 0), (0, 0), (1, 1), (1, 1)),
                 mode='reflect').astype(f16)

    in_maps = []
    for i in range(NCORES):
        sl = slice(BPC * i, BPC * (i + 1))
        x0 = np.ascontiguousarray(xp[sl].reshape(BPC * C, WP, HP))
        x0l = np.ascontiguousarray(xpl[sl].reshape(BPC * C, WP, HP))
        x8 = np.stack([x0[:, :, 0:H], x0[:, :, 2:2 + H]], axis=1)
        x8 = np.ascontiguousarray(x8).astype(e4m3)
        g2 = GLw[:, sl].reshape(STEPS, BPC, NG, NPIX).transpose(0, 2, 1, 3)
        glc = np.ascontiguousarray(
            np.broadcast_to(g2[:, :, :, None, :],
                            (STEPS, NG, BPC, 64, NPIX))
            .reshape(STEPS, NG, 128, NPIX)).astype(bf16)
        in_maps.append({
            "x0": x0, "x0l": x0l, "x8": x8,
            "wt8": wt8, "wtb": wtb, "wf": wf, "bi": bi, "gl": glc,
        })
    return in_maps


def _run(inputs, trace=False, trace_kwargs=None):
    from concourse.bass_utils import run_bass_kernel_spmd
    if "nc" not in _nc_cache:
        _nc_cache["nc"] = _build()
    nc = _nc_cache["nc"]
    in_maps = _host_pack(
        inputs["x"], inputs["w_conv1"], inputs["w_conv2"], inputs["w_hidden"],
        inputs["b_hidden"], inputs["w_final"], inputs["rand_vals"])
    kwargs = {}
    if trace:
        kwargs["trace"] = True
        if trace_kwargs:
            kwargs.update(trace_kwargs)
    res = run_bass_kernel_spmd(nc, in_maps, core_ids=list(range(NCORES)), **kwargs)
    outs = []
    for i in range(NCORES):
        o = res.results[i]["out"].astype(np.float32).reshape(BPC, C, W, H)
        outs.append(o.transpose(0, 3, 2, 1))         # -> [b, H, W, C]
    full = np.concatenate(outs, axis=0).astype(np.float32)
    return full, res


def kernel(**inputs) -> np.ndarray:
    steps = int(np.asarray(inputs.get("steps", STEPS)))
    assert steps == STEPS, f"kernel compiled for {STEPS} steps, got {steps}"
    out, _ = _run(inputs)
    return out


# revision 17
# speedup vs baseline: 4.4317x; 1.0156x over previous
"""Trainium2 Bass kernel for nn_BasicNCAModel (neural cellular automaton).

Strategy (pure data parallelism, batch 16 -> 2 images per core x 8 cores):

* State layout per core: [128 partitions = 2 images x 64 channels,
  130 x 130 reflect-padded grid] in SBUF fp16, ping-pong buffered.
* The two depthwise 3x3 convs are folded into the hidden matmul:
  h = relu(sum_tap E_tap @ x_shift(tap) + b) with E_tap[256, 64].
* fp8 DoubleRow: NPAIRS of the 9 taps run as e4m3 DoubleRow matmuls that
  contract TWO taps per instruction (2 fp8 weights/PE cell).  The moving
  operand comes from packed fp8 state copies x8[b-idx][130 rows, 128 cols]
  (H-stride 128, no column halo) so a group's 512 pixels are contiguous;
  the K-pair stride (delta = d_bidx*16640 + d_a*128) is 16B-aligned as
  DoubleRow requires.  Remaining taps (incl. the W0-carrying center) stay
  fp16 for accuracy; per-hidden-row scales (folded into bias and w_final)
  keep the e4m3 weights in range.
* No per-step barrier: halo cols are refreshed per group, halo rows right
  after the first/last group of each step, so consecutive steps pipeline
  on the PE without HAM re-throttle.
* Stochastic fire gate (pre-merged with the static life mask on the host)
  is broadcast per group on GpSimd and applied on DVE; the masked initial
  state x0*L feeds the first update so step 0 needs no life tiles.
"""
import sys
sys.path.insert(0, '/opt/trn_rl_repo')

import numpy as np

B, H, W, C = 16, 128, 128, 64
HID = 256
STEPS = 8
NCORES = 8
BPC = B // NCORES            # images per core = 2
WP, HP = W + 2, H + 2        # padded grid 130 x 130
RPG = 4                      # W-rows per group
NPIX = RPG * H               # 512 pixels per matmul tile
NG = W // RPG                # 32 groups per step
CSZ = WP * H                 # 16640 elements per packed fp8 copy

NPAIRS = 3                   # fp8 DoubleRow tap pairs (0, 2 or 3)

# tap schedule: pairs are e4m3 DoubleRow (2 taps/MM); singles + center fp16.
# b-copies: bidx 0 <-> b=0 (cols 0:128 of padded state), 1 <-> b=2 (cols 2:130)
BIDX = {0: 0, 2: 1}
if NPAIRS == 3:
    PAIRS = [((0, 0), (1, 0)), ((2, 0), (0, 2)), ((1, 2), (2, 2))]
    SINGLES = [(0, 1), (2, 1), (1, 1)]
elif NPAIRS == 2:
    PAIRS = [((0, 0), (1, 0)), ((1, 2), (2, 2))]
    SINGLES = [(2, 0), (0, 2), (0, 1), (2, 1), (1, 1)]
else:
    PAIRS = []
    SINGLES = [(a, b) for a in range(3) for b in range(3)]
NS = len(SINGLES)

_nc_cache = {}


def _build():
    import concourse.bacc as bacc
    import concourse.mybir as mybir
    import concourse.tile as tile
    from concourse.bass import AP

    F32 = mybir.dt.float32
    F16 = mybir.dt.float16
    BF16 = mybir.dt.bfloat16
    F8 = mybir.dt.float8e4
    AF = mybir.ActivationFunctionType
    ALU = mybir.AluOpType
    DR = mybir.MatmulPerfMode.DoubleRow

    nc = bacc.Bacc("TRN2", target_bir_lowering=False, debug=False,
                   enable_asserts=False, num_devices=NCORES)

    X0 = nc.dram_tensor("x0", [128, WP, HP], F16, kind="ExternalInput")
    X0L = nc.dram_tensor("x0l", [128, WP, HP], F16, kind="ExternalInput")
    X8 = nc.dram_tensor("x8", [128, 2, WP, H], F8, kind="ExternalInput")
    WT8 = nc.dram_tensor("wt8", [128, max(NPAIRS, 1), 2, 2, 128], F8,
                         kind="ExternalInput")
    WTB = nc.dram_tensor("wtb", [128, NS, 2, 128], F16, kind="ExternalInput")
    WF = nc.dram_tensor("wf", [128, 2, 64], BF16, kind="ExternalInput")
    BI = nc.dram_tensor("bi", [128, 2], F32, kind="ExternalInput")
    GL = nc.dram_tensor("gl", [STEPS, NG, 128, NPIX], BF16, kind="ExternalInput")
    OUT = nc.dram_tensor("out", [128, W, H], F16, kind="ExternalOutput")

    with tile.TileContext(nc) as tc:
        with tc.tile_pool(name="const", bufs=1) as cp, \
             tc.tile_pool(name="hbuf", bufs=2) as hp, \
             tc.tile_pool(name="gbuf", bufs=3) as gp, \
             tc.tile_pool(name="ph", bufs=1, space="PSUM") as php, \
             tc.tile_pool(name="pdx", bufs=2, space="PSUM") as pdxp:

            xA = cp.tile([128, WP, HP], F16, tag="xA")
            xB = cp.tile([128, WP, HP], F16, tag="xB")
            x0l = cp.tile([128, WP, HP], F16, tag="x0l")
            x8A = cp.tile([128, 2, WP, H], F8, tag="x8A")
            x8B = cp.tile([128, 2, WP, H], F8, tag="x8B")
            wt8 = cp.tile([128, max(NPAIRS, 1), 2, 2, 128], F8, tag="wt8")
            wtb = cp.tile([128, NS, 2, 128], F16, tag="wtb")
            wf = cp.tile([128, 2, 64], BF16, tag="wf")
            bi = cp.tile([128, 2], F32, tag="bi")

            nc.sync.dma_start(wt8[:], WT8[:])
            nc.sync.dma_start(wtb[:], WTB[:])
            nc.sync.dma_start(wf[:], WF[:])
            nc.sync.dma_start(bi[:], BI[:])
            NCH = 10
            for c in range(NCH):
                r0, r1 = (WP * c) // NCH, (WP * (c + 1)) // NCH
                nc.sync.dma_start(xA[:, r0:r1, :], X0[:, r0:r1, :])
                nc.sync.dma_start(x8A[:, :, r0:r1, :], X8[:, :, r0:r1, :])
            for c in range(4):
                r0, r1 = (WP * c) // 4, (WP * (c + 1)) // 4
                nc.sync.dma_start(x0l[:, r0:r1, :], X0L[:, r0:r1, :])

            def dr_rhs(x8s, img, w0, t1, t2):
                """[64, 2@delta, 512@1] moving AP for a DoubleRow tap pair."""
                (a1, b1), (a2, b2) = t1, t2
                delta = (BIDX[b2] - BIDX[b1]) * CSZ + (a2 - a1) * H
                assert delta > 0 and delta % 16 == 0, (t1, t2, delta)
                base = x8s[img * 64:(img + 1) * 64, BIDX[b1],
                           w0 + a1:w0 + a1 + RPG, :]
                ap = [list(base.ap[0]), [delta, 2], [1, NPIX]]
                return AP(base.tensor, base.offset, ap)

            def emit_tail(p):
                """mm2 + gate + state update + fp8 copy refresh for a
                finished group (issued one group later: PE never stalls)."""
                hA, hB, gate, gateB, xs, xd, x8d, w0, t = p
                first, last = t == 0, t == STEPS - 1
                dx = pdxp.tile([128, NPIX], F32, tag="dx")
                for k in range(2):
                    nc.tensor.matmul(dx[0:64, :], wf[:, k, :], hA[:, k, :],
                                     start=k == 0, stop=k == 1,
                                     skip_group_check=True)
                    nc.tensor.matmul(dx[64:128, :], wf[:, k, :], hB[:, k, :],
                                     start=k == 0, stop=k == 1,
                                     skip_group_check=True,
                                     tile_position=(0, 64))
                tg = hp.tile([128, NPIX], F16, tag="tg")
                nc.vector.tensor_tensor(tg[:], dx[:], gate[:], ALU.mult)
                tg3 = tg[:].rearrange("p (a b) -> p a b", a=RPG)
                rows = slice(w0 + 1, w0 + 1 + RPG)
                src = x0l if first else xs
                nc.vector.tensor_tensor(xd[:, rows, 1:1 + H], tg3,
                                        src[:, rows, 1:1 + H], ALU.add)
                # packed fp8 b-copies; reflect halo columns read directly
                # (interior cols 1..128 only: nothing reads state cols 0/129)
                nc.scalar.copy(x8d[:, 0, rows, 1:H], xd[:, rows, 1:H])
                nc.scalar.copy(x8d[:, 0, rows, 0:1], xd[:, rows, 2:3])
                nc.vector.tensor_copy(x8d[:, 1, rows, 0:H - 1],
                                      xd[:, rows, 2:1 + H])
                nc.vector.tensor_copy(x8d[:, 1, rows, H - 1:H],
                                      xd[:, rows, H - 1:H])
                if w0 == 0:
                    # reflect row halos for next step's first group
                    nc.vector.tensor_copy(xd[:, 0, 1:1 + H], xd[:, 2, 1:1 + H])
                    nc.scalar.copy(x8d[:, 0, 0, 1:H], xd[:, 2, 1:H])
                    nc.scalar.copy(x8d[:, 0, 0, 0:1], xd[:, 2, 2:3])
                    nc.vector.tensor_copy(x8d[:, 1, 0, 0:H - 1], xd[:, 2, 2:1 + H])
                    nc.vector.tensor_copy(x8d[:, 1, 0, H - 1:H],
                                          xd[:, 2, H - 1:H])
                if w0 == W - RPG:
                    r = WP - 3
                    nc.vector.tensor_copy(xd[:, WP - 1, 1:1 + H], xd[:, r, 1:1 + H])
                    nc.scalar.copy(x8d[:, 0, WP - 1, 1:H], xd[:, r, 1:H])
                    nc.scalar.copy(x8d[:, 0, WP - 1, 0:1], xd[:, r, 2:3])
                    nc.vector.tensor_copy(x8d[:, 1, WP - 1, 0:H - 1],
                                          xd[:, r, 2:1 + H])
                    nc.vector.tensor_copy(x8d[:, 1, WP - 1, H - 1:H],
                                          xd[:, r, H - 1:H])
                if last:
                    nc.sync.dma_start(OUT[:, w0:w0 + RPG, :],
                                      xd[:, rows, 1:1 + H])

            pend = None
            for t in range(STEPS):
                xs, xd = (xA, xB) if t % 2 == 0 else (xB, xA)
                x8s, x8d = (x8A, x8B) if t % 2 == 0 else (x8B, x8A)
                for g in range(NG):
                    w0 = RPG * g

                    gate = gp.tile([128, NPIX], BF16, tag="gate")
                    gateB = gate
                    nc.sync.dma_start(gate[:], GL[t, g])

                    phs = [[php.tile([128, NPIX], F32, tag=f"ph{im}{m}",
                                     name=f"ph{im}{m}")
                            for m in range(2)] for im in range(2)]
                    hA = hp.tile([128, 2, NPIX], BF16, tag="hA")
                    hB = hp.tile([128, 2, NPIX], BF16, tag="hB")
                    for m in range(2):
                        nmm = NPAIRS + NS
                        mi = 0
                        for pi, (t1, t2) in enumerate(PAIRS):
                            st, sp = mi == 0, mi == nmm - 1
                            for im in range(2):
                                nc.tensor.matmul(
                                    phs[im][m][:], wt8[im * 64:(im + 1) * 64, pi, m],
                                    dr_rhs(x8s, im, w0, t1, t2),
                                    start=st, stop=sp, perf_mode=DR,
                                    skip_group_check=True)
                            mi += 1
                        for si, (a, b) in enumerate(SINGLES):
                            st, sp = mi == 0, mi == nmm - 1
                            for im in range(2):
                                rhs = xs[im * 64:(im + 1) * 64,
                                         w0 + a:w0 + a + RPG, b:b + H]
                                nc.tensor.matmul(
                                    phs[im][m][:], wtb[im * 64:(im + 1) * 64, si, m],
                                    rhs, start=st, stop=sp,
                                    skip_group_check=True)
                            mi += 1
                        # relu + bias, PSUM -> SBUF bf16 (3 on ACT, 1 on DVE)
                        nc.scalar.activation(hA[:, m, :], phs[0][m][:], AF.Relu,
                                             bias=bi[:, m:m + 1])
                        if m == 0:
                            nc.scalar.activation(hB[:, m, :], phs[1][m][:],
                                                 AF.Relu, bias=bi[:, m:m + 1])
                        else:
                            nc.vector.tensor_scalar(
                                out=hB[:, m, :], in0=phs[1][m][:],
                                scalar1=bi[:, m:m + 1], scalar2=0.0,
                                op0=ALU.add, op1=ALU.max)

                    if pend is not None:
                        emit_tail(pend)
                    pend = (hA, hB, gate, gateB, xs, xd, x8d, w0, t)

            emit_tail(pend)

    nc.compile()
    return nc


def _host_pack(x, w_conv1, w_conv2, w_hidden, b_hidden, w_final, rand_vals):
    import ml_dtypes
    bf16 = ml_dtypes.bfloat16
    f16 = np.float16
    e4m3 = ml_dtypes.float8_e4m3

    Wh = np.asarray(w_hidden, np.float64)            # [256, 192]
    w1 = np.asarray(w_conv1, np.float64)[:, 0]       # [64, 3, 3]
    w2 = np.asarray(w_conv2, np.float64)[:, 0]

    E = {}
    for a in range(3):
        for b in range(3):
            Et = Wh[:, 64:128] * w1[None, :, a, b] + Wh[:, 128:192] * w2[None, :, a, b]
            if (a, b) == (1, 1):
                Et = Et + Wh[:, 0:64]
            E[(a, b)] = Et                            # [256, 64]

    fp8taps = [tp for pr in PAIRS for tp in pr]
    if fp8taps:
        rowmax = np.max(np.stack([np.abs(E[tp]) for tp in fp8taps]), axis=(0, 2))
        s = np.clip(224.0 / np.maximum(rowmax, 1e-6), 0.25, 4096.0)   # [256]
    else:
        s = np.ones(HID)

    wt8 = np.zeros((128, max(NPAIRS, 1), 2, 2, 128), np.float32)
    for pi, (t1, t2) in enumerate(PAIRS):
        for ko, tp in enumerate((t1, t2)):
            Es = E[tp] * s[:, None]                   # [256, 64]
            for m in range(2):
                lhsT = Es[128 * m:128 * (m + 1), :].T                 # [64, 128]
                wt8[0:64, pi, m, ko, :] = lhsT
                wt8[64:128, pi, m, ko, :] = lhsT
    wt8 = wt8.astype(e4m3)

    wtb = np.zeros((128, NS, 2, 128), np.float32)
    for si, tp in enumerate(SINGLES):
        Es = E[tp] * s[:, None]
        for m in range(2):
            lhsT = Es[128 * m:128 * (m + 1), :].T
            wtb[0:64, si, m, :] = lhsT
            wtb[64:128, si, m, :] = lhsT
    wtb = wtb.astype(f16)

    bv = np.asarray(b_hidden, np.float64) * s
    bi = np.stack([bv[0:128], bv[128:256]], axis=1).astype(np.float32)

    wfz = np.asarray(w_final, np.float64).copy()     # [64, 256]
    wfz[0:4, :] = 0.0                                # immutable image channels
    wfT = (wfz / s[None, :]).T                       # [256, 64]
    wf = np.ascontiguousarray(
        np.stack([wfT[0:128], wfT[128:256]], axis=1)).astype(bf16)

    # life mask is static: channel-0 updates masked out -> life == (x0 > 0)
    Lhw = np.asarray(x)[..., 0] > 0                  # [B, H, W]
    Lwh = np.ascontiguousarray(Lhw.transpose(0, 2, 1))   # [B, W, H]
    G = np.asarray(rand_vals)[..., 0] > 0.5          # [S, B, H, W]
    GLw = G.transpose(0, 1, 3, 2) & Lwh[None]        # [S, B, W, H]

    x_chw = np.asarray(x, np.float32).transpose(0, 3, 2, 1)      # [B, C, W, H]
    xp = np.pad(x_chw, ((0, 0), (0, 0), (1, 1), (1, 1)), mode='reflect')
    xp = xp.astype(f16)
    xl = x_chw * Lwh[:, None].astype(np.float32)
    xpl = np.pad(xl, ((0, 0), (0, 0), (1, 1), (1, 1)),
                 mode='reflect').astype(f16)

    in_maps = []
    for i in range(NCORES):
        sl = slice(BPC * i, BPC * (i + 1))
        x0 = np.ascontiguousarray(xp[sl].reshape(BPC * C, WP, HP))
        x0l = np.ascontiguousarray(xpl[sl].reshape(BPC * C, WP, HP))
        x8 = np.stack([x0[:, :, 0:H], x0[:, :, 2:2 + H]], axis=1)
        x8 = np.ascontiguousarray(x8).astype(e4m3)
        g2 = GLw[:, sl].reshape(STEPS, BPC, NG, NPIX).transpose(0, 2, 1, 3)
        glc = np.ascontiguousarray(
            np.broadcast_to(g2[:, :, :, None, :],
                            (STEPS, NG, BPC, 64, NPIX))
            .reshape(STEPS, NG, 128, NPIX)).astype(bf16)
        in_maps.append({
            "x0": x0, "x0l": x0l, "x8": x8,
            "wt8": wt8, "wtb": wtb, "wf": wf, "bi": bi, "gl": glc,
        })
    return in_maps


def _run(inputs, trace=False, trace_kwargs=None):
    from concourse.bass_utils import run_bass_kernel_spmd
    if "nc" not in _nc_cache:
        _nc_cache["nc"] = _build()
    nc = _nc_cache["nc"]
    in_maps = _host_pack(
        inputs["x"], inputs["w_conv1"], inputs["w_conv2"], inputs["w_hidden"],
        inputs["b_hidden"], inputs["w_final"], inputs["rand_vals"])
    kwargs = {}
    if trace:
        kwargs["trace"] = True
        if trace_kwargs:
            kwargs.update(trace_kwargs)
    res = run_bass_kernel_spmd(nc, in_maps, core_ids=list(range(NCORES)), **kwargs)
    outs = []
    for i in range(NCORES):
        o = res.results[i]["out"].astype(np.float32).reshape(BPC, C, W, H)
        outs.append(o.transpose(0, 3, 2, 1))         # -> [b, H, W, C]
    full = np.concatenate(outs, axis=0).astype(np.float32)
    return full, res


def kernel(**inputs) -> np.ndarray:
    steps = int(np.asarray(inputs.get("steps", STEPS)))
    assert steps == STEPS, f"kernel compiled for {STEPS} steps, got {steps}"
    out, _ = _run(inputs)
    return out


# revision 25
# speedup vs baseline: 4.4525x; 1.0047x over previous
"""Trainium2 Bass kernel for nn_BasicNCAModel (neural cellular automaton).

Strategy (pure data parallelism, batch 16 -> 2 images per core x 8 cores):

* State layout per core: [128 partitions = 2 images x 64 channels,
  130 x 130 reflect-padded grid] in SBUF fp16, ping-pong buffered.
* The two depthwise 3x3 convs are folded into the hidden matmul:
  h = relu(sum_tap E_tap @ x_shift(tap) + b) with E_tap[256, 64].
* fp8 DoubleRow: NPAIRS of the 9 taps run as e4m3 DoubleRow matmuls that
  contract TWO taps per instruction (2 fp8 weights/PE cell).  The moving
  operand comes from packed fp8 state copies x8[b-idx][130 rows, 128 cols]
  (H-stride 128, no column halo) so a group's 512 pixels are contiguous;
  the K-pair stride (delta = d_bidx*16640 + d_a*128) is 16B-aligned as
  DoubleRow requires.  Remaining taps (incl. the W0-carrying center) stay
  fp16 for accuracy; per-hidden-row scales (folded into bias and w_final)
  keep the e4m3 weights in range.
* No per-step barrier: halo cols are refreshed per group, halo rows right
  after the first/last group of each step, so consecutive steps pipeline
  on the PE without HAM re-throttle.
* Stochastic fire gate (pre-merged with the static life mask on the host)
  is broadcast per group on GpSimd and applied on DVE; the masked initial
  state x0*L feeds the first update so step 0 needs no life tiles.
"""
import sys
sys.path.insert(0, '/opt/trn_rl_repo')

import numpy as np

B, H, W, C = 16, 128, 128, 64
HID = 256
STEPS = 8
NCORES = 8
BPC = B // NCORES            # images per core = 2
WP, HP = W + 2, H + 2        # padded grid 130 x 130
RPG = 4                      # W-rows per group
NPIX = RPG * H               # 512 pixels per matmul tile
NG = W // RPG                # 32 groups per step
CSZ = WP * H                 # 16640 elements per packed fp8 copy

NPAIRS = 4                   # fp8 DoubleRow tap pairs (0, 2, 3 or 4)

# tap schedule: pairs are e4m3 DoubleRow (2 taps/MM); singles + center fp16.
# b-copies: bidx b <-> cols b:b+128 of the padded state
NB = 3 if NPAIRS == 4 else 2
BIDX = {0: 0, 1: 1, 2: 2} if NB == 3 else {0: 0, 2: 1}
if NPAIRS == 4:
    PAIRS = [((0, 0), (1, 0)), ((2, 0), (0, 1)), ((2, 1), (0, 2)),
             ((1, 2), (2, 2))]
    SINGLES = [(1, 1)]
elif NPAIRS == 3:
    PAIRS = [((0, 0), (1, 0)), ((2, 0), (0, 2)), ((1, 2), (2, 2))]
    SINGLES = [(0, 1), (2, 1), (1, 1)]
elif NPAIRS == 2:
    PAIRS = [((0, 0), (1, 0)), ((1, 2), (2, 2))]
    SINGLES = [(2, 0), (0, 2), (0, 1), (2, 1), (1, 1)]
else:
    PAIRS = []
    SINGLES = [(a, b) for a in range(3) for b in range(3)]
NS = len(SINGLES)

_nc_cache = {}


def _build():
    import concourse.bacc as bacc
    import concourse.mybir as mybir
    import concourse.tile as tile
    from concourse.bass import AP

    F32 = mybir.dt.float32
    F16 = mybir.dt.float16
    BF16 = mybir.dt.bfloat16
    F8 = mybir.dt.float8e4
    AF = mybir.ActivationFunctionType
    ALU = mybir.AluOpType
    DR = mybir.MatmulPerfMode.DoubleRow

    nc = bacc.Bacc("TRN2", target_bir_lowering=False, debug=False,
                   enable_asserts=False, num_devices=NCORES)

    X0 = nc.dram_tensor("x0", [128, WP, HP], F16, kind="ExternalInput")
    X8 = nc.dram_tensor("x8", [128, NB, WP, H], F8, kind="ExternalInput")
    LG = nc.dram_tensor("lg", [NG, 128, NPIX], BF16, kind="ExternalInput")
    WT8 = nc.dram_tensor("wt8", [128, max(NPAIRS, 1), 2, 2, 128], F8,
                         kind="ExternalInput")
    WTB = nc.dram_tensor("wtb", [128, NS, 2, 128], F16, kind="ExternalInput")
    WF = nc.dram_tensor("wf", [128, 2, 64], BF16, kind="ExternalInput")
    BI = nc.dram_tensor("bi", [128, 2], F32, kind="ExternalInput")
    GL = nc.dram_tensor("gl", [STEPS, NG, 128, NPIX], BF16, kind="ExternalInput")
    OUT = nc.dram_tensor("out", [128, W, H], F16, kind="ExternalOutput")

    with tile.TileContext(nc) as tc:
        with tc.tile_pool(name="const", bufs=1) as cp, \
             tc.tile_pool(name="hbuf", bufs=2) as hp, \
             tc.tile_pool(name="gbuf", bufs=3) as gp, \
             tc.tile_pool(name="ph", bufs=1, space="PSUM") as php, \
             tc.tile_pool(name="pdx", bufs=2, space="PSUM") as pdxp:

            xA = cp.tile([128, WP, HP], F16, tag="xA")
            xB = cp.tile([128, WP, HP], F16, tag="xB")
            x8A = cp.tile([128, NB, WP, H], F8, tag="x8A")
            x8B = cp.tile([128, NB, WP, H], F8, tag="x8B")
            wt8 = cp.tile([128, max(NPAIRS, 1), 2, 2, 128], F8, tag="wt8")
            wtb = cp.tile([128, NS, 2, 128], F16, tag="wtb")
            wf = cp.tile([128, 2, 64], BF16, tag="wf")
            bi = cp.tile([128, 2], F32, tag="bi")

            nc.sync.dma_start(wt8[:], WT8[:])
            nc.sync.dma_start(wtb[:], WTB[:])
            nc.sync.dma_start(wf[:], WF[:])
            nc.sync.dma_start(bi[:], BI[:])
            NCH = 10
            for c in range(NCH):
                r0, r1 = (WP * c) // NCH, (WP * (c + 1)) // NCH
                nc.sync.dma_start(xA[:, r0:r1, :], X0[:, r0:r1, :])
                nc.sync.dma_start(x8A[:, :, r0:r1, :], X8[:, :, r0:r1, :])

            def dr_rhs(x8s, img, w0, t1, t2):
                """[64, 2@delta, 512@1] moving AP for a DoubleRow tap pair."""
                (a1, b1), (a2, b2) = t1, t2
                delta = (BIDX[b2] - BIDX[b1]) * CSZ + (a2 - a1) * H
                assert delta > 0 and delta % 16 == 0, (t1, t2, delta)
                base = x8s[img * 64:(img + 1) * 64, BIDX[b1],
                           w0 + a1:w0 + a1 + RPG, :]
                ap = [list(base.ap[0]), [delta, 2], [1, NPIX]]
                return AP(base.tensor, base.offset, ap)

            def emit_tail(p):
                """mm2 + gate + state update + fp8 copy refresh for a
                finished group (issued one group later: PE never stalls)."""
                hA, hB, gate, gateB, life, xs, xd, x8d, w0, t = p
                first, last = t == 0, t == STEPS - 1
                dx = pdxp.tile([128, NPIX], F32, tag="dx")
                for k in range(2):
                    nc.tensor.matmul(dx[0:64, :], wf[:, k, :], hA[:, k, :],
                                     start=k == 0, stop=k == 1,
                                     skip_group_check=True)
                    nc.tensor.matmul(dx[64:128, :], wf[:, k, :], hB[:, k, :],
                                     start=k == 0, stop=k == 1,
                                     skip_group_check=True,
                                     tile_position=(0, 64))
                tg = hp.tile([128, NPIX], F16, tag="tg")
                nc.vector.tensor_tensor(tg[:], dx[:], gate[:], ALU.mult)
                tg3 = tg[:].rearrange("p (a b) -> p a b", a=RPG)
                rows = slice(w0 + 1, w0 + 1 + RPG)
                if first:
                    # x1 = x0*L + dx*GL (GL already includes L)
                    tl = hp.tile([128, NPIX], F16, tag="tl")
                    nc.vector.tensor_tensor(
                        tl[:].rearrange("p (a b) -> p a b", a=RPG),
                        xs[:, rows, 1:1 + H], life[:].rearrange(
                            "p (a b) -> p a b", a=RPG), ALU.mult)
                    src3 = tl[:].rearrange("p (a b) -> p a b", a=RPG)
                else:
                    src3 = xs[:, rows, 1:1 + H]
                nc.gpsimd.tensor_tensor(xd[:, rows, 1:1 + H], tg3, src3,
                                        ALU.add)

                # packed fp8 b-copies; reflect halo columns read directly
                # (interior cols 1..128 only: nothing reads state cols 0/129)
                def casts(dst_w, src_w):
                    nc.scalar.copy(x8d[:, 0, dst_w, 1:H], xd[:, src_w, 1:H])
                    nc.scalar.copy(x8d[:, 0, dst_w, 0:1], xd[:, src_w, 2:3])
                    if NB == 3:
                        nc.vector.tensor_copy(x8d[:, 1, dst_w, :],
                                              xd[:, src_w, 1:1 + H])
                    bl = NB - 1
                    nc.vector.tensor_copy(x8d[:, bl, dst_w, 0:H - 1],
                                          xd[:, src_w, 2:1 + H])
                    nc.vector.tensor_copy(x8d[:, bl, dst_w, H - 1:H],
                                          xd[:, src_w, H - 1:H])

                casts(rows, rows)
                if w0 == 0:
                    # reflect row halos for next step's first group
                    if NS > 1:
                        nc.vector.tensor_copy(xd[:, 0, 1:1 + H],
                                              xd[:, 2, 1:1 + H])
                    casts(0, 2)
                if w0 == W - RPG:
                    if NS > 1:
                        nc.vector.tensor_copy(xd[:, WP - 1, 1:1 + H],
                                              xd[:, WP - 3, 1:1 + H])
                    casts(WP - 1, WP - 3)
                if last:
                    nc.sync.dma_start(OUT[:, w0:w0 + RPG, :],
                                      xd[:, rows, 1:1 + H])

            pend = None
            for t in range(STEPS):
                xs, xd = (xA, xB) if t % 2 == 0 else (xB, xA)
                x8s, x8d = (x8A, x8B) if t % 2 == 0 else (x8B, x8A)
                for g in range(NG):
                    w0 = RPG * g

                    gate = gp.tile([128, NPIX], BF16, tag="gate")
                    gateB = gate
                    nc.sync.dma_start(gate[:], GL[t, g])
                    life = None
                    if t == 0:
                        life = gp.tile([128, NPIX], BF16, tag="life")
                        nc.sync.dma_start(life[:], LG[g])

                    phs = [[php.tile([128, NPIX], F32, tag=f"ph{im}{m}",
                                     name=f"ph{im}{m}")
                            for m in range(2)] for im in range(2)]
                    hA = hp.tile([128, 2, NPIX], BF16, tag="hA")
                    hB = hp.tile([128, 2, NPIX], BF16, tag="hB")
                    for m in range(2):
                        nmm = NPAIRS + NS
                        mi = 0
                        for pi, (t1, t2) in enumerate(PAIRS):
                            st, sp = mi == 0, mi == nmm - 1
                            for im in range(2):
                                nc.tensor.matmul(
                                    phs[im][m][:], wt8[im * 64:(im + 1) * 64, pi, m],
                                    dr_rhs(x8s, im, w0, t1, t2),
                                    start=st, stop=sp, perf_mode=DR,
                                    skip_group_check=True)
                            mi += 1
                        for si, (a, b) in enumerate(SINGLES):
                            st, sp = mi == 0, mi == nmm - 1
                            for im in range(2):
                                rhs = xs[im * 64:(im + 1) * 64,
                                         w0 + a:w0 + a + RPG, b:b + H]
                                nc.tensor.matmul(
                                    phs[im][m][:], wtb[im * 64:(im + 1) * 64, si, m],
                                    rhs, start=st, stop=sp,
                                    skip_group_check=True)
                            mi += 1
                        # relu + bias, PSUM -> SBUF bf16 (3 on ACT, 1 on DVE)
                        nc.scalar.activation(hA[:, m, :], phs[0][m][:], AF.Relu,
                                             bias=bi[:, m:m + 1])
                        if m == 0:
                            nc.scalar.activation(hB[:, m, :], phs[1][m][:],
                                                 AF.Relu, bias=bi[:, m:m + 1])
                        else:
                            nc.vector.tensor_scalar(
                                out=hB[:, m, :], in0=phs[1][m][:],
                                scalar1=bi[:, m:m + 1], scalar2=0.0,
                                op0=ALU.add, op1=ALU.max)

                    if pend is not None:
                        emit_tail(pend)
                    pend = (hA, hB, gate, gateB, life, xs, xd, x8d, w0, t)

            emit_tail(pend)

    nc.compile()
    return nc


def _host_pack(x, w_conv1, w_conv2, w_hidden, b_hidden, w_final, rand_vals):
    import ml_dtypes
    bf16 = ml_dtypes.bfloat16
    f16 = np.float16
    e4m3 = ml_dtypes.float8_e4m3

    Wh = np.asarray(w_hidden, np.float64)            # [256, 192]
    w1 = np.asarray(w_conv1, np.float64)[:, 0]       # [64, 3, 3]
    w2 = np.asarray(w_conv2, np.float64)[:, 0]

    E = {}
    for a in range(3):
        for b in range(3):
            Et = Wh[:, 64:128] * w1[None, :, a, b] + Wh[:, 128:192] * w2[None, :, a, b]
            if (a, b) == (1, 1):
                Et = Et + Wh[:, 0:64]
            E[(a, b)] = Et                            # [256, 64]

    fp8taps = [tp for pr in PAIRS for tp in pr]
    if fp8taps:
        rowmax = np.max(np.stack([np.abs(E[tp]) for tp in fp8taps]), axis=(0, 2))
        s = np.clip(224.0 / np.maximum(rowmax, 1e-6), 0.25, 4096.0)   # [256]
    else:
        s = np.ones(HID)

    wt8 = np.zeros((128, max(NPAIRS, 1), 2, 2, 128), np.float32)
    for pi, (t1, t2) in enumerate(PAIRS):
        for ko, tp in enumerate((t1, t2)):
            Es = E[tp] * s[:, None]                   # [256, 64]
            for m in range(2):
                lhsT = Es[128 * m:128 * (m + 1), :].T                 # [64, 128]
                wt8[0:64, pi, m, ko, :] = lhsT
                wt8[64:128, pi, m, ko, :] = lhsT
    wt8 = wt8.astype(e4m3)

    wtb = np.zeros((128, NS, 2, 128), np.float32)
    for si, tp in enumerate(SINGLES):
        Es = E[tp] * s[:, None]
        for m in range(2):
            lhsT = Es[128 * m:128 * (m + 1), :].T
            wtb[0:64, si, m, :] = lhsT
            wtb[64:128, si, m, :] = lhsT
    wtb = wtb.astype(f16)

    bv = np.asarray(b_hidden, np.float64) * s
    bi = np.stack([bv[0:128], bv[128:256]], axis=1).astype(np.float32)

    wfz = np.asarray(w_final, np.float64).copy()     # [64, 256]
    wfz[0:4, :] = 0.0                                # immutable image channels
    wfT = (wfz / s[None, :]).T                       # [256, 64]
    wf = np.ascontiguousarray(
        np.stack([wfT[0:128], wfT[128:256]], axis=1)).astype(bf16)

    # life mask is static: channel-0 updates masked out -> life == (x0 > 0)
    Lhw = np.asarray(x)[..., 0] > 0                  # [B, H, W]
    Lwh = np.ascontiguousarray(Lhw.transpose(0, 2, 1))   # [B, W, H]
    G = np.asarray(rand_vals)[..., 0] > 0.5          # [S, B, H, W]
    GLw = G.transpose(0, 1, 3, 2) & Lwh[None]        # [S, B, W, H]

    x_chw = np.asarray(x, np.float32).transpose(0, 3, 2, 1)      # [B, C, W, H]
    xp = np.pad(x_chw, ((0, 0), (0, 0), (1, 1), (1, 1)), mode='reflect')
    xp = xp.astype(f16)

    bvals = sorted(BIDX, key=lambda b: BIDX[b])
    in_maps = []
    for i in range(NCORES):
        sl = slice(BPC * i, BPC * (i + 1))
        x0 = np.ascontiguousarray(xp[sl].reshape(BPC * C, WP, HP))
        x8 = np.stack([x0[:, :, b:b + H] for b in bvals], axis=1)
        x8 = np.ascontiguousarray(x8).astype(e4m3)
        g2 = GLw[:, sl].reshape(STEPS, BPC, NG, NPIX).transpose(0, 2, 1, 3)
        glc = np.ascontiguousarray(
            np.broadcast_to(g2[:, :, :, None, :],
                            (STEPS, NG, BPC, 64, NPIX))
            .reshape(STEPS, NG, 128, NPIX)).astype(bf16)
        l2 = Lwh[sl].reshape(BPC, NG, NPIX).transpose(1, 0, 2)
        lgc = np.ascontiguousarray(
            np.broadcast_to(l2[:, :, None, :], (NG, BPC, 64, NPIX))
            .reshape(NG, 128, NPIX)).astype(bf16)
        in_maps.append({
            "x0": x0, "x8": x8, "lg": lgc,
            "wt8": wt8, "wtb": wtb, "wf": wf, "bi": bi, "gl": glc,
        })
    return in_maps


def _run(inputs, trace=False, trace_kwargs=None):
    from concourse.bass_utils import run_bass_kernel_spmd
    if "nc" not in _nc_cache:
        _nc_cache["nc"] = _build()
    nc = _nc_cache["nc"]
    in_maps = _host_pack(
        inputs["x"], inputs["w_conv1"], inputs["w_conv2"], inputs["w_hidden"],
        inputs["b_hidden"], inputs["w_final"], inputs["rand_vals"])
    kwargs = {}
    if trace:
        kwargs["trace"] = True
        if trace_kwargs:
            kwargs.update(trace_kwargs)
    res = run_bass_kernel_spmd(nc, in_maps, core_ids=list(range(NCORES)), **kwargs)
    outs = []
    for i in range(NCORES):
        o = res.results[i]["out"].astype(np.float32).reshape(BPC, C, W, H)
        outs.append(o.transpose(0, 3, 2, 1))         # -> [b, H, W, C]
    full = np.concatenate(outs, axis=0).astype(np.float32)
    return full, res


def kernel(**inputs) -> np.ndarray:
    steps = int(np.asarray(inputs.get("steps", STEPS)))
    assert steps == STEPS, f"kernel compiled for {STEPS} steps, got {steps}"
    out, _ = _run(inputs)
    return out


# revision 27
# speedup vs baseline: 4.5784x; 1.0283x over previous
"""Trainium2 Bass kernel for nn_BasicNCAModel (neural cellular automaton).

Strategy (pure data parallelism, batch 16 -> 2 images per core x 8 cores):

* State layout per core: [128 partitions = 2 images x 64 channels,
  130 x 130 reflect-padded grid] in SBUF fp16, ping-pong buffered.
* The two depthwise 3x3 convs are folded into the hidden matmul:
  h = relu(sum_tap E_tap @ x_shift(tap) + b) with E_tap[256, 64].
* fp8 DoubleRow: NPAIRS of the 9 taps run as e4m3 DoubleRow matmuls that
  contract TWO taps per instruction (2 fp8 weights/PE cell).  The moving
  operand comes from packed fp8 state copies x8[b-idx][130 rows, 128 cols]
  (H-stride 128, no column halo) so a group's 512 pixels are contiguous;
  the K-pair stride (delta = d_bidx*16640 + d_a*128) is 16B-aligned as
  DoubleRow requires.  Remaining taps (incl. the W0-carrying center) stay
  fp16 for accuracy; per-hidden-row scales (folded into bias and w_final)
  keep the e4m3 weights in range.
* No per-step barrier: halo cols are refreshed per group, halo rows right
  after the first/last group of each step, so consecutive steps pipeline
  on the PE without HAM re-throttle.
* Stochastic fire gate (pre-merged with the static life mask on the host)
  is broadcast per group on GpSimd and applied on DVE; the masked initial
  state x0*L feeds the first update so step 0 needs no life tiles.
"""
import sys
sys.path.insert(0, '/opt/trn_rl_repo')

import numpy as np

B, H, W, C = 16, 128, 128, 64
HID = 256
STEPS = 8
NCORES = 8
BPC = B // NCORES            # images per core = 2
WP, HP = W + 2, H + 2        # padded grid 130 x 130
RPG = 4                      # W-rows per group
NPIX = RPG * H               # 512 pixels per matmul tile
NG = W // RPG                # 32 groups per step
CSZ = WP * H                 # 16640 elements per packed fp8 copy

NPAIRS = 4                   # fp8 DoubleRow tap pairs (0, 2, 3 or 4)

# tap schedule: pairs are e4m3 DoubleRow (2 taps/MM); singles + center fp16.
# b-copies: bidx b <-> cols b:b+128 of the padded state
NB = 3 if NPAIRS == 4 else 2
BIDX = {0: 0, 1: 1, 2: 2} if NB == 3 else {0: 0, 2: 1}
if NPAIRS == 4:
    PAIRS = [((0, 0), (1, 0)), ((2, 0), (0, 1)), ((2, 1), (0, 2)),
             ((1, 2), (2, 2))]
    SINGLES = [(1, 1)]
elif NPAIRS == 3:
    PAIRS = [((0, 0), (1, 0)), ((2, 0), (0, 2)), ((1, 2), (2, 2))]
    SINGLES = [(0, 1), (2, 1), (1, 1)]
elif NPAIRS == 2:
    PAIRS = [((0, 0), (1, 0)), ((1, 2), (2, 2))]
    SINGLES = [(2, 0), (0, 2), (0, 1), (2, 1), (1, 1)]
else:
    PAIRS = []
    SINGLES = [(a, b) for a in range(3) for b in range(3)]
NS = len(SINGLES)

_nc_cache = {}


def _build():
    import concourse.bacc as bacc
    import concourse.mybir as mybir
    import concourse.tile as tile
    from concourse.bass import AP

    F32 = mybir.dt.float32
    F16 = mybir.dt.float16
    BF16 = mybir.dt.bfloat16
    F8 = mybir.dt.float8e4
    AF = mybir.ActivationFunctionType
    ALU = mybir.AluOpType
    DR = mybir.MatmulPerfMode.DoubleRow

    nc = bacc.Bacc("TRN2", target_bir_lowering=False, debug=False,
                   enable_asserts=False, num_devices=NCORES)

    X0 = nc.dram_tensor("x0", [128, WP, HP], F16, kind="ExternalInput")
    X8 = nc.dram_tensor("x8", [128, NB, WP, H], F8, kind="ExternalInput")
    LG = nc.dram_tensor("lg", [NG, 128, NPIX], BF16, kind="ExternalInput")
    WT8 = nc.dram_tensor("wt8", [128, max(NPAIRS, 1), 2, 2, 128], F8,
                         kind="ExternalInput")
    WTB = nc.dram_tensor("wtb", [128, NS, 2, 128], F16, kind="ExternalInput")
    WF = nc.dram_tensor("wf", [128, 2, 64], BF16, kind="ExternalInput")
    BI = nc.dram_tensor("bi", [128, 2], F32, kind="ExternalInput")
    GL = nc.dram_tensor("gl", [STEPS, NG, 128, NPIX], BF16, kind="ExternalInput")
    OUT = nc.dram_tensor("out", [128, W, H], F16, kind="ExternalOutput")

    with tile.TileContext(nc) as tc:
        with tc.tile_pool(name="const", bufs=1) as cp, \
             tc.tile_pool(name="hbuf", bufs=2) as hp, \
             tc.tile_pool(name="gbuf", bufs=3) as gp, \
             tc.tile_pool(name="ph", bufs=1, space="PSUM") as php, \
             tc.tile_pool(name="pdx", bufs=2, space="PSUM") as pdxp:

            xA = cp.tile([128, WP, HP], F16, tag="xA")
            xB = cp.tile([128, WP, HP], F16, tag="xB")
            x8A = cp.tile([128, NB, WP, H], F8, tag="x8A")
            x8B = cp.tile([128, NB, WP, H], F8, tag="x8B")
            wt8 = cp.tile([128, max(NPAIRS, 1), 2, 2, 128], F8, tag="wt8")
            wtb = cp.tile([128, NS, 2, 128], F16, tag="wtb")
            wf = cp.tile([128, 2, 64], BF16, tag="wf")
            bi = cp.tile([128, 2], F32, tag="bi")

            nc.sync.dma_start(wt8[:], WT8[:])
            nc.sync.dma_start(wtb[:], WTB[:])
            nc.sync.dma_start(wf[:], WF[:])
            nc.sync.dma_start(bi[:], BI[:])
            NCH = 10
            for c in range(NCH):
                r0, r1 = (WP * c) // NCH, (WP * (c + 1)) // NCH
                nc.sync.dma_start(xA[:, r0:r1, :], X0[:, r0:r1, :])
                nc.sync.dma_start(x8A[:, :, r0:r1, :], X8[:, :, r0:r1, :])

            def dr_rhs(x8s, img, w0, t1, t2):
                """[64, 2@delta, 512@1] moving AP for a DoubleRow tap pair."""
                (a1, b1), (a2, b2) = t1, t2
                delta = (BIDX[b2] - BIDX[b1]) * CSZ + (a2 - a1) * H
                assert delta > 0 and delta % 16 == 0, (t1, t2, delta)
                base = x8s[img * 64:(img + 1) * 64, BIDX[b1],
                           w0 + a1:w0 + a1 + RPG, :]
                ap = [list(base.ap[0]), [delta, 2], [1, NPIX]]
                return AP(base.tensor, base.offset, ap)

            def emit_tail(p):
                """mm2 + gate + state update + fp8 copy refresh for a
                finished group (issued one group later: PE never stalls)."""
                hA, hB, gate, gateB, life, xs, xd, x8d, w0, t = p
                first, last = t == 0, t == STEPS - 1
                dx = pdxp.tile([128, NPIX], F32, tag="dx")
                for k in range(2):
                    nc.tensor.matmul(dx[0:64, :], wf[:, k, :], hA[:, k, :],
                                     start=k == 0, stop=k == 1,
                                     skip_group_check=True)
                    nc.tensor.matmul(dx[64:128, :], wf[:, k, :], hB[:, k, :],
                                     start=k == 0, stop=k == 1,
                                     skip_group_check=True,
                                     tile_position=(0, 64))
                tg = hp.tile([128, NPIX], F16, tag="tg")
                nc.vector.tensor_tensor(tg[:], dx[:], gate[:], ALU.mult)
                tg3 = tg[:].rearrange("p (a b) -> p a b", a=RPG)
                rows = slice(w0 + 1, w0 + 1 + RPG)
                if first:
                    # x1 = x0*L + dx*GL (GL already includes L)
                    tl = hp.tile([128, NPIX], F16, tag="tl")
                    nc.vector.tensor_tensor(
                        tl[:].rearrange("p (a b) -> p a b", a=RPG),
                        xs[:, rows, 1:1 + H], life[:].rearrange(
                            "p (a b) -> p a b", a=RPG), ALU.mult)
                    src3 = tl[:].rearrange("p (a b) -> p a b", a=RPG)
                else:
                    src3 = xs[:, rows, 1:1 + H]
                nc.vector.tensor_tensor(xd[:, rows, 1:1 + H], tg3, src3,
                                        ALU.add)

                if last:
                    nc.sync.dma_start(OUT[:, w0:w0 + RPG, :],
                                      xd[:, rows, 1:1 + H])
                    return  # final state: fp8 copies never read again

                # packed fp8 b-copies; reflect halo columns read directly
                # (interior cols 1..128 only: nothing reads state cols 0/129).
                # Edge columns (b=0 pos 0 / b=2 pos 127) are batched per
                # half-step in the main loop; halo rows get theirs inline.
                def casts(dst_w, src_w, tiny):
                    nc.scalar.copy(x8d[:, 0, dst_w, 1:H], xd[:, src_w, 1:H])
                    if tiny:
                        nc.scalar.copy(x8d[:, 0, dst_w, 0:1], xd[:, src_w, 2:3])
                    if NB == 3:
                        nc.vector.tensor_copy(x8d[:, 1, dst_w, :],
                                              xd[:, src_w, 1:1 + H])
                    bl = NB - 1
                    nc.vector.tensor_copy(x8d[:, bl, dst_w, 0:H - 1],
                                          xd[:, src_w, 2:1 + H])
                    if tiny:
                        nc.vector.tensor_copy(x8d[:, bl, dst_w, H - 1:H],
                                              xd[:, src_w, H - 1:H])

                casts(rows, rows, False)
                if w0 == 0:
                    # reflect row halos for next step's first group
                    if NS > 1:
                        nc.vector.tensor_copy(xd[:, 0, 1:1 + H],
                                              xd[:, 2, 1:1 + H])
                    casts(0, 2, True)
                if w0 == W - RPG:
                    if NS > 1:
                        nc.vector.tensor_copy(xd[:, WP - 1, 1:1 + H],
                                              xd[:, WP - 3, 1:1 + H])
                    casts(WP - 1, WP - 3, True)

            pend = None
            for t in range(STEPS):
                xs, xd = (xA, xB) if t % 2 == 0 else (xB, xA)
                x8s, x8d = (x8A, x8B) if t % 2 == 0 else (x8B, x8A)
                for g in range(NG):
                    w0 = RPG * g

                    gate = gp.tile([128, NPIX], BF16, tag="gate")
                    gateB = gate
                    nc.sync.dma_start(gate[:], GL[t, g])
                    life = None
                    if t == 0:
                        life = gp.tile([128, NPIX], BF16, tag="life")
                        nc.sync.dma_start(life[:], LG[g])

                    phs = [[php.tile([128, NPIX], F32, tag=f"ph{im}{m}",
                                     name=f"ph{im}{m}")
                            for m in range(2)] for im in range(2)]
                    hA = hp.tile([128, 2, NPIX], BF16, tag="hA")
                    hB = hp.tile([128, 2, NPIX], BF16, tag="hB")
                    for m in range(2):
                        nmm = NPAIRS + NS
                        mi = 0
                        for pi, (t1, t2) in enumerate(PAIRS):
                            st, sp = mi == 0, mi == nmm - 1
                            for im in range(2):
                                nc.tensor.matmul(
                                    phs[im][m][:], wt8[im * 64:(im + 1) * 64, pi, m],
                                    dr_rhs(x8s, im, w0, t1, t2),
                                    start=st, stop=sp, perf_mode=DR,
                                    skip_group_check=True)
                            mi += 1
                        for si, (a, b) in enumerate(SINGLES):
                            st, sp = mi == 0, mi == nmm - 1
                            for im in range(2):
                                rhs = xs[im * 64:(im + 1) * 64,
                                         w0 + a:w0 + a + RPG, b:b + H]
                                nc.tensor.matmul(
                                    phs[im][m][:], wtb[im * 64:(im + 1) * 64, si, m],
                                    rhs, start=st, stop=sp,
                                    skip_group_check=True)
                            mi += 1
                        # relu + bias, PSUM -> SBUF bf16 (3 on ACT, 1 on DVE)
                        nc.scalar.activation(hA[:, m, :], phs[0][m][:], AF.Relu,
                                             bias=bi[:, m:m + 1])
                        if m == 0:
                            nc.scalar.activation(hB[:, m, :], phs[1][m][:],
                                                 AF.Relu, bias=bi[:, m:m + 1])
                        else:
                            nc.vector.tensor_scalar(
                                out=hB[:, m, :], in0=phs[1][m][:],
                                scalar1=bi[:, m:m + 1], scalar2=0.0,
                                op0=ALU.add, op1=ALU.max)

                    if pend is not None:
                        emit_tail(pend)
                    pend = (hA, hB, gate, gateB, life, xs, xd, x8d, w0, t)

                    def tiny_batch(x8t, xt, r0, r1):
                        nc.scalar.copy(x8t[:, 0, r0:r1, 0:1], xt[:, r0:r1, 2:3])
                        nc.vector.tensor_copy(x8t[:, NB - 1, r0:r1, H - 1:H],
                                              xt[:, r0:r1, H - 1:H])

                    if g == 17 and t < STEPS - 1:
                        tiny_batch(x8d, xd, 1, NG * 2 + 1)
                    if g == 1 and 1 <= t:
                        tiny_batch(x8s, xs, NG * 2 + 1, WP - 1)

            emit_tail(pend)

    nc.compile()
    return nc


def _host_pack(x, w_conv1, w_conv2, w_hidden, b_hidden, w_final, rand_vals):
    import ml_dtypes
    bf16 = ml_dtypes.bfloat16
    f16 = np.float16
    e4m3 = ml_dtypes.float8_e4m3

    Wh = np.asarray(w_hidden, np.float64)            # [256, 192]
    w1 = np.asarray(w_conv1, np.float64)[:, 0]       # [64, 3, 3]
    w2 = np.asarray(w_conv2, np.float64)[:, 0]

    E = {}
    for a in range(3):
        for b in range(3):
            Et = Wh[:, 64:128] * w1[None, :, a, b] + Wh[:, 128:192] * w2[None, :, a, b]
            if (a, b) == (1, 1):
                Et = Et + Wh[:, 0:64]
            E[(a, b)] = Et                            # [256, 64]

    fp8taps = [tp for pr in PAIRS for tp in pr]
    if fp8taps:
        rowmax = np.max(np.stack([np.abs(E[tp]) for tp in fp8taps]), axis=(0, 2))
        s = np.clip(224.0 / np.maximum(rowmax, 1e-6), 0.25, 4096.0)   # [256]
    else:
        s = np.ones(HID)

    wt8 = np.zeros((128, max(NPAIRS, 1), 2, 2, 128), np.float32)
    for pi, (t1, t2) in enumerate(PAIRS):
        for ko, tp in enumerate((t1, t2)):
            Es = E[tp] * s[:, None]                   # [256, 64]
            for m in range(2):
                lhsT = Es[128 * m:128 * (m + 1), :].T                 # [64, 128]
                wt8[0:64, pi, m, ko, :] = lhsT
                wt8[64:128, pi, m, ko, :] = lhsT
    wt8 = wt8.astype(e4m3)

    wtb = np.zeros((128, NS, 2, 128), np.float32)
    for si, tp in enumerate(SINGLES):
        Es = E[tp] * s[:, None]
        for m in range(2):
            lhsT = Es[128 * m:128 * (m + 1), :].T
            wtb[0:64, si, m, :] = lhsT
            wtb[64:128, si, m, :] = lhsT
    wtb = wtb.astype(f16)

    bv = np.asarray(b_hidden, np.float64) * s
    bi = np.stack([bv[0:128], bv[128:256]], axis=1).astype(np.float32)

    wfz = np.asarray(w_final, np.float64).copy()     # [64, 256]
    wfz[0:4, :] = 0.0                                # immutable image channels
    wfT = (wfz / s[None, :]).T                       # [256, 64]
    wf = np.ascontiguousarray(
        np.stack([wfT[0:128], wfT[128:256]], axis=1)).astype(bf16)

    # life mask is static: channel-0 updates masked out -> life == (x0 > 0)
    Lhw = np.asarray(x)[..., 0] > 0                  # [B, H, W]
    Lwh = np.ascontiguousarray(Lhw.transpose(0, 2, 1))   # [B, W, H]
    G = np.asarray(rand_vals)[..., 0] > 0.5          # [S, B, H, W]
    GLw = G.transpose(0, 1, 3, 2) & Lwh[None]        # [S, B, W, H]

    x_chw = np.asarray(x, np.float32).transpose(0, 3, 2, 1)      # [B, C, W, H]
    xp = np.pad(x_chw, ((0, 0), (0, 0), (1, 1), (1, 1)), mode='reflect')
    xp = xp.astype(f16)

    bvals = sorted(BIDX, key=lambda b: BIDX[b])
    in_maps = []
    for i in range(NCORES):
        sl = slice(BPC * i, BPC * (i + 1))
        x0 = np.ascontiguousarray(xp[sl].reshape(BPC * C, WP, HP))
        x8 = np.stack([x0[:, :, b:b + H] for b in bvals], axis=1)
        x8 = np.ascontiguousarray(x8).astype(e4m3)
        g2 = GLw[:, sl].reshape(STEPS, BPC, NG, NPIX).transpose(0, 2, 1, 3)
        glc = np.ascontiguousarray(
            np.broadcast_to(g2[:, :, :, None, :],
                            (STEPS, NG, BPC, 64, NPIX))
            .reshape(STEPS, NG, 128, NPIX)).astype(bf16)
        l2 = Lwh[sl].reshape(BPC, NG, NPIX).transpose(1, 0, 2)
        lgc = np.ascontiguousarray(
            np.broadcast_to(l2[:, :, None, :], (NG, BPC, 64, NPIX))
            .reshape(NG, 128, NPIX)).astype(bf16)
        in_maps.append({
            "x0": x0, "x8": x8, "lg": lgc,
            "wt8": wt8, "wtb": wtb, "wf": wf, "bi": bi, "gl": glc,
        })
    return in_maps


def _run(inputs, trace=False, trace_kwargs=None):
    from concourse.bass_utils import run_bass_kernel_spmd
    if "nc" not in _nc_cache:
        _nc_cache["nc"] = _build()
    nc = _nc_cache["nc"]
    in_maps = _host_pack(
        inputs["x"], inputs["w_conv1"], inputs["w_conv2"], inputs["w_hidden"],
        inputs["b_hidden"], inputs["w_final"], inputs["rand_vals"])
    kwargs = {}
    if trace:
        kwargs["trace"] = True
        if trace_kwargs:
            kwargs.update(trace_kwargs)
    res = run_bass_kernel_spmd(nc, in_maps, core_ids=list(range(NCORES)), **kwargs)
    outs = []
    for i in range(NCORES):
        o = res.results[i]["out"].astype(np.float32).reshape(BPC, C, W, H)
        outs.append(o.transpose(0, 3, 2, 1))         # -> [b, H, W, C]
    full = np.concatenate(outs, axis=0).astype(np.float32)
    return full, res


def kernel(**inputs) -> np.ndarray:
    steps = int(np.asarray(inputs.get("steps", STEPS)))
    assert steps == STEPS, f"kernel compiled for {STEPS} steps, got {steps}"
    out, _ = _run(inputs)
    return out
